# revision 15
# baseline (speedup 1.0000x reference)
"""DeepSeek sparse attention on 8 Trainium2 NeuronCores.

Head-sharded (2 heads/core). v2 schedule — single Act table set
({Exp, Copy, Sign} all live in exp_and_others -> zero table swaps),
selection/attention software-pipelined:

  - P0: indexer projection I = hs @ A via bf16 hi/lo 3-pass matmul; Q/K/V
    projections bf16, stationary-grouped loops.
  - selection per (h,i): X = iq@ik^T via 96-row hi/lo-stacked bf16 matmul;
    top-32 threshold per query via 16-subset DVE MAX8 + 4 rounds
    max8/match_replace; mask m = Sign(X - t32 + eps) in {-1,+1} on the
    Scalar engine, DMA-transposed to [s,q].
  - attention in [s,q]: e = exp(S^T) (Scalar); w' = e*m (GpSimd TT);
    AV accumulates BOTH e and w' into one PSUM group so av equals
    2*(masked AV) — the ones-row normalization absorbs the factor 2.
    The w'-AV matmuls are emitted one step late so the PE queue never
    head-of-line blocks on the GpSimd multiply.
  - schedule: selection is the DVE-paced spine; attention steps fill
    PE/Act/GpSimd under it: attn(0,half0) under sel-0 slots 8-15,
    attn(0,half1)+attn(1,half0) under sel-1, attn(1,half1) as the tail.
  - out_proj partial per core; host sums the 8 partials.
"""
import sys

sys.path.insert(0, '/opt/trn_rl_repo')
sys.path.insert(0, '/opt/pypackages')

import numpy as np
import ml_dtypes

BF16 = ml_dtypes.bfloat16

B, T, D = 1, 2048, 1024
H, DH, DI, KSEL = 16, 64, 32, 32
NCORES = 8
HPC = H // NCORES
NT = T // 128               # 16 query/key tiles
NK = D // 128               # 8 contraction chunks


MARGIN = 1e-5               # inclusion margin on the threshold

_COMPILED = {}


def _install_drain_patch():
    import concourse.mybir as mybir
    from concourse.tile import TileContext
    from concourse.vector_clock import ScopedClock

    if getattr(TileContext, "_dsa_patched", False):
        return

    def _patched(self, tick_clock, wait_clock):
        nc = self.nc
        drain_inst = nc.sync.drain()
        wait_clock.add_sem_waits(
            drain_inst.ins, ScopedClock({None: tick_clock.global_clock})
        )
        si = drain_inst.ins.sync_info
        waits = list(si.on_wait or []) if si is not None else []
        if len(waits) > 1:
            drain_inst.ins.sync_info = mybir.SyncInfo(
                on_wait=waits[:1], on_update=list(si.on_update or [])
            )
            for i in range(1, len(waits)):
                extra = nc.sync.drain()
                extra.ins.sync_info = mybir.SyncInfo(
                    on_wait=waits[i:i + 1], on_update=[]
                )
        nc.all_engine_barrier()
        assert self.sems is not None
        popped = nc._tile_sem_poison_stack.pop()
        assert popped is self._sem_poison
        nc.clear_and_free_semaphores(list(self.sems.allocated().values()))
        nc.all_engine_barrier()

    TileContext._drain_and_barrier = _patched
    TileContext._dsa_patched = True


def _split_excess_waits(nc, limit=1):
    """walrus in this container rejects instructions with more sync waits
    than the ISA struct encodes; hoist excess waits onto standalone
    EventSemaphore instructions on the same engine, inserted just before."""
    import concourse.mybir as mybir

    n_new = 0
    for bb in nc.main_func.blocks:
        insts = bb.instructions
        i = 0
        while i < len(insts):
            ins = insts[i]
            si = ins.sync_info
            waits = list(si.on_wait or []) if si is not None else []
            if len(waits) > limit:
                ins.sync_info = mybir.SyncInfo(
                    on_wait=waits[:limit], on_update=list(si.on_update or []))
                pos = i
                for j in range(limit, len(waits), limit):
                    n_new += 1
                    w = mybir.InstEventSemaphore(
                        name=f"WSPLIT-{n_new}", ins=[], outs=[])
                    w.engine = ins.engine
                    w.sync_info = mybir.SyncInfo(
                        on_wait=waits[j:j + limit], on_update=[])
                    nc.register_instruction(w, overwrite=True)
                    insts.insert(pos, w)
                    pos += 1
                    i += 1
            i += 1
    return n_new


def _build_module():
    import concourse.bass as bass
    import concourse.mybir as mybir
    from concourse.tile import TileContext

    _install_drain_patch()
    dt = mybir.dt
    nc = bass.Bass()

    hsT_hi = nc.declare_dram_parameter("hsT_hi", [D, T], dt.bfloat16, isOutput=False)
    hsT_lo = nc.declare_dram_parameter("hsT_lo", [D, T], dt.bfloat16, isOutput=False)
    A_hi = nc.declare_dram_parameter("A_hi", [D, 128], dt.bfloat16, isOutput=False)
    A_lo = nc.declare_dram_parameter("A_lo", [D, 128], dt.bfloat16, isOutput=False)
    Wqk_h0 = nc.declare_dram_parameter("Wqk_h0", [D, 128], dt.bfloat16, isOutput=False)
    Wqk_h1 = nc.declare_dram_parameter("Wqk_h1", [D, 128], dt.bfloat16, isOutput=False)
    Wv_cat = nc.declare_dram_parameter("Wv_cat", [D, 128], dt.bfloat16, isOutput=False)
    WoT_cat = nc.declare_dram_parameter("WoT_cat", [128, D], dt.bfloat16, isOutput=False)
    out_part = nc.declare_dram_parameter("out_part", [T, D], dt.float32, isOutput=True)

    Sign = mybir.ActivationFunctionType.Sign
    Exp = mybir.ActivationFunctionType.Exp
    Copy = mybir.ActivationFunctionType.Copy
    MUL = mybir.AluOpType.mult
    SUB = mybir.AluOpType.subtract
    ADD = mybir.AluOpType.add

    with TileContext(nc) as tc:
        with tc.tile_pool(name="state", bufs=1) as st:
            IqSs = [st.tile([96, T], dt.bfloat16, tag=f"IqS{h}", name=f"IqS{h}")
                    for h in range(2)]
            IkSs = [st.tile([96, T], dt.bfloat16, tag=f"IkS{h}", name=f"IkS{h}")
                    for h in range(2)]
            QT = st.tile([128, T], dt.bfloat16, tag="QT")
            KT = st.tile([128, T], dt.bfloat16, tag="KT")
            VP = st.tile([128, NT, 2, 65], dt.bfloat16, tag="VP")
            ATcatT = st.tile([128, T], dt.bfloat16, tag="ATcatT")
            wo = st.tile([128, D], dt.bfloat16, tag="wo")
            nc.sync.dma_start(out=wo[:], in_=WoT_cat[:])

            # ================= P0: projections =================
            with tc.tile_pool(name="hsbp", bufs=1) as hp, \
                 tc.tile_pool(name="p0w", bufs=1) as p0w, \
                 tc.tile_pool(name="p0p", bufs=1, space="PSUM") as p0p:
                hsb = hp.tile([128, NK, T], dt.bfloat16, tag="hsb")
                hslo = hp.tile([128, NK, T], dt.bfloat16, tag="hslo")
                a_h = p0w.tile([128, NK, 128], dt.bfloat16, tag="a_h")
                a_l = p0w.tile([128, NK, 128], dt.bfloat16, tag="a_l")
                qk0_w = p0w.tile([128, NK, 128], dt.bfloat16, tag="qk0_w")
                qk1_w = p0w.tile([128, NK, 128], dt.bfloat16, tag="qk1_w")
                v_w = p0w.tile([128, NK, 128], dt.bfloat16, tag="v_w")
                nc.sync.dma_start(out=a_h[:], in_=A_hi[:].rearrange("(c p) m -> p c m", p=128))
                nc.sync.dma_start(out=a_l[:], in_=A_lo[:].rearrange("(c p) m -> p c m", p=128))
                nc.sync.dma_start(out=qk0_w[:], in_=Wqk_h0[:].rearrange("(c p) m -> p c m", p=128))
                nc.sync.dma_start(out=qk1_w[:], in_=Wqk_h1[:].rearrange("(c p) m -> p c m", p=128))
                nc.sync.dma_start(out=v_w[:], in_=Wv_cat[:].rearrange("(c p) m -> p c m", p=128))
                for k in range(NK):
                    nc.sync.dma_start(out=hsb[:, k, :], in_=hsT_hi[128 * k:128 * k + 128, :])
                    nc.sync.dma_start(out=hslo[:, k, :], in_=hsT_lo[128 * k:128 * k + 128, :])

                ip = p0p.tile([128, T], dt.float32, tag="ip")
                qp = p0p.tile([128, T], dt.float32, tag="qp")
                # pass A: I (hi/lo 3-pass) + QK_h0, LDW-grouped per stationary
                for k in range(NK):
                    for n in range(4):
                        sl = slice(512 * n, 512 * n + 512)
                        nc.tensor.matmul(ip[:, sl], a_h[:, k, :], hsb[:, k, sl],
                                         start=(k == 0), stop=False)
                    for n in range(4):
                        sl = slice(512 * n, 512 * n + 512)
                        nc.tensor.matmul(ip[:, sl], a_h[:, k, :], hslo[:, k, sl],
                                         start=False, stop=False)
                    for n in range(4):
                        sl = slice(512 * n, 512 * n + 512)
                        nc.tensor.matmul(ip[:, sl], a_l[:, k, :], hsb[:, k, sl],
                                         start=False, stop=(k == NK - 1))
                    for n in range(4):
                        sl = slice(512 * n, 512 * n + 512)
                        nc.tensor.matmul(qp[:, sl], qk0_w[:, k, :], hsb[:, k, sl],
                                         start=(k == 0), stop=(k == NK - 1))
                # evac indexer projections: contraction-stacked hi/lo
                for h in range(2):
                    iqr = ip[32 * h:32 * h + 32, :]
                    ikr = ip[64 + 32 * h:64 + 32 * h + 32, :]
                    IqS, IkS = IqSs[h], IkSs[h]
                    nc.scalar.copy(out=IqS[0:32, :], in_=iqr)
                    nc.vector.tensor_copy(IqS[32:64, :], IqS[0:32, :])
                    nc.vector.tensor_tensor(out=IqS[64:96, :], in0=iqr,
                                            in1=IqS[0:32, :], op=SUB)
                    nc.scalar.copy(out=IkS[0:32, :], in_=ikr)
                    nc.vector.tensor_tensor(out=IkS[32:64, :], in0=ikr,
                                            in1=IkS[0:32, :], op=SUB)
                    nc.vector.tensor_copy(IkS[64:96, :], IkS[0:32, :])
                nc.scalar.copy(out=QT[0:64, :], in_=qp[0:64, :])
                nc.scalar.copy(out=KT[0:64, :], in_=qp[64:128, :])

                # pass B: QK_h1 + V
                qp1 = p0p.tile([128, T], dt.float32, tag="ip")  # reuse slot
                vp_ps = p0p.tile([128, T], dt.float32, tag="qp")
                for k in range(NK):
                    for n in range(4):
                        sl = slice(512 * n, 512 * n + 512)
                        nc.tensor.matmul(qp1[:, sl], qk1_w[:, k, :], hsb[:, k, sl],
                                         start=(k == 0), stop=(k == NK - 1))
                    for n in range(4):
                        sl = slice(512 * n, 512 * n + 512)
                        nc.tensor.matmul(vp_ps[:, sl], v_w[:, k, :], hsb[:, k, sl],
                                         start=(k == 0), stop=(k == NK - 1))
                nc.scalar.copy(out=QT[64:128, :], in_=qp1[0:64, :])
                nc.scalar.copy(out=KT[64:128, :], in_=qp1[64:128, :])
                # V: rows 0:64 = V_h0^T [dh, s], 64:128 = V_h1^T; DMA-transpose
                vt_b = p0w.tile([128, T], dt.bfloat16, tag="vt_b")
                nc.scalar.copy(out=vt_b[:], in_=vp_ps[:])
                vq = p0w.tile([128, NT, 128], dt.bfloat16, tag="vq")
                nc.sync.dma_start_transpose(out=vq[:], in_=vt_b[:])
                for j in range(NT):
                    for h in range(2):
                        nc.scalar.copy(out=VP[:, j, h, 0:64],
                                       in_=vq[:, j, 64 * h:64 * h + 64])
                nc.vector.memset(VP[:, :, :, 64:65], 1.0)

            # ================= selection + attention pipeline =================
            with tc.tile_pool(name="mtp", bufs=2) as mtp, \
                 tc.tile_pool(name="pa", bufs=2) as pa, \
                 tc.tile_pool(name="pms", bufs=2) as pms, \
                 tc.tile_pool(name="pe_", bufs=2) as pe_, \
                 tc.tile_pool(name="pc", bufs=1) as pc:
                pools = {}

                mts = [mtp.tile([128, NT, NT, 128], dt.bfloat16, tag="mt",
                                name=f"mt{h}") for h in range(2)]
                ats = [pc.tile([128, T], dt.bfloat16, tag=f"at{h}", name=f"at{h}")
                       for h in range(2)]

                def emit_sel_compute(h, i):
                    """X matmuls + candidate top-8s + rounds -> negt; returns
                    (xps, negt) for the deferred mask pass"""
                    cand = pa.tile([128, 128], dt.float32, tag="cand")
                    xps = []
                    for half in range(2):
                        xp = pools["pxs"].tile([128, 1024], dt.float32, tag="xps",
                                      name=f"xp{h}_{i}_{half}")
                        xps.append(xp)
                        for n in range(2):
                            sl = slice(512 * n, 512 * n + 512)
                            c0 = 1024 * half + 512 * n
                            qsl = slice(128 * i, 128 * i + 128)
                            nc.tensor.matmul(xp[:, sl], IqSs[h][:, qsl],
                                             IkSs[h][:, c0:c0 + 512])
                        v8 = xp[:].rearrange("p (s l) -> p l s", l=8)
                        for j in range(8):
                            nc.vector.max(out=cand[:, 64 * half + 8 * j:64 * half + 8 * j + 8],
                                          in_=v8[:, j, :])
                    mx = pa.tile([128, 8], dt.float32, tag="mx")
                    for r in range(4):
                        nc.vector.max(out=mx[:], in_=cand[:])
                        if r < 3:
                            nc.vector.match_replace(out=cand[:], in_to_replace=mx[:],
                                                    in_values=cand[:], imm_value=-1e30)
                    # negt = -t32 + margin; m = Sign(X + negt) in {-1,+1}
                    # (Sign lives in exp_and_others with Exp/Copy: no table swap)
                    negt = pa.tile([128, 1], dt.float32, tag="negt")
                    nc.vector.tensor_scalar(negt[:], mx[:, 7:8], -1.0,
                                            scalar2=MARGIN, op0=MUL, op1=ADD)
                    return xps, negt

                def emit_sel_mask(h, i, xps, negt):
                    """Sign masks + transpose. Emitted AFTER the slot's attn
                    exps so the Act queue never head-of-line blocks on negt."""
                    mt = mts[h]
                    ms = pms.tile([128, T], dt.bfloat16, tag="ms", name=f"ms{h}_{i}")
                    for half in range(2):
                        nc.scalar.activation(
                            out=ms[:, 1024 * half:1024 * half + 1024],
                            in_=xps[half][:], func=Sign, bias=negt[:])
                    nc.sync.dma_start_transpose(out=mt[:, :, i, :], in_=ms[:])

                # per-stream state: current av tile + the lagged w'-AV
                # emissions (one pending per 512-query quarter)
                avst = {"av": None, "pending": []}

                def flush_wav(last=False):
                    """emit the lagged w'-AV matmuls for the previous step"""
                    for av, wt, h, j, q in avst["pending"]:
                        sl = slice(512 * q, 512 * q + 512)
                        nc.tensor.matmul(av[:, sl], VP[:, j, h, :], wt[:],
                                         start=False, stop=last)
                    avst["pending"] = []

                def emit_attn(h, half, j, mul_dve=False):
                    """attention step at 512-query-quarter granularity:
                    sp_q = K_j^T Q_quarter (1 matmul, 1 PSUM bank, bufs=2 so
                    consecutive steps pipeline); e_q = exp(sp_q);
                    w'_q = e_q * sign-mask (GpSimd TT, or DVE TT in the tail
                    where DVE is idle); av += VP@e and += VP@w' so av equals
                    2*(masked AV) — the ones-row normalization absorbs it.
                    The w'-AV matmuls are emitted one step late to avoid PE
                    queue head-of-line blocking on the mask multiply."""
                    mt = mts[h]
                    sps, es = [], []
                    for q in range(2):
                        sp = pools["psp"].tile([128, 512], dt.float32, tag="sp",
                                               name=f"sp{h}_{half}_{j}_{q}")
                        c0 = 1024 * half + 512 * q
                        nc.tensor.matmul(sp[:],
                                         KT[64 * h:64 * h + 64, 128 * j:128 * j + 128],
                                         QT[64 * h:64 * h + 64, c0:c0 + 512])
                        sps.append(sp)
                    for q in range(2):
                        e = pe_.tile([128, 512], dt.bfloat16, tag="e", name="e")
                        nc.scalar.activation(out=e[:], in_=sps[q][:], func=Exp)
                        es.append(e)
                    msl = mt[:, j, 8 * half:8 * half + 8, :].rearrange("p a b -> p (a b)")
                    wts = []
                    for q in range(2):
                        wt = pe_.tile([128, 512], dt.bfloat16, tag="w", name="w")
                        sl = slice(512 * q, 512 * q + 512)
                        if mul_dve:
                            nc.vector.tensor_tensor(out=wt[:], in0=es[q][:],
                                                    in1=msl[:, sl], op=MUL)
                        else:
                            nc.gpsimd.tensor_tensor(out=wt[:], in0=es[q][:],
                                                    in1=msl[:, sl], op=MUL)
                        wts.append(wt)
                    if j == 0:
                        avst["av"] = pools["pav"].tile([65, 1024], dt.float32, tag="av",
                                                       name=f"av{h}_{half}")
                    av = avst["av"]
                    for q in range(2):
                        sl = slice(512 * q, 512 * q + 512)
                        nc.tensor.matmul(av[:, sl], VP[:, j, h, :], es[q][:],
                                         start=(j == 0), stop=False)
                    flush_wav()
                    avst["pending"] = [(av, wts[q], h, j, q) for q in range(2)]

                def emit_attn_end(h, half):
                    flush_wav(last=True)
                    nc.scalar.copy(out=ats[h][0:65, 1024 * half:1024 * half + 1024],
                                   in_=avst["av"][:])

                def emit_C(h):
                    """normalize + build transposed attn rows of ATcatT"""
                    at = ats[h]
                    atq = pc.tile([128, NT, 128], dt.bfloat16, tag="atq")
                    nc.sync.dma_start_transpose(out=atq[:], in_=at[:])
                    scrall = pc.tile([128, NT, 128], dt.bfloat16, tag="scrall")
                    rds = pa.tile([128, NT], dt.float32, tag="rds")
                    nc.vector.reciprocal(
                        rds[:], atq[:, :, 64:65].rearrange("p a b -> p (a b)"))
                    for i in range(NT):
                        nc.scalar.activation(out=scrall[:, i, 0:64],
                                             in_=atq[:, i, 0:64], func=Copy,
                                             scale=rds[:, i:i + 1])
                    tmpT = pc.tile([128, NT, 128], dt.bfloat16, tag="tmpT")
                    nc.sync.dma_start_transpose(out=tmpT[:], in_=scrall[:])
                    nc.vector.tensor_copy(ATcatT[64 * h:64 * h + 64, :],
                                          tmpT[0:64, :, :].rearrange("p a b -> p (a b)"))

                # ---- issue schedule ----
                with tc.tile_pool(name="pxs", bufs=2, space="PSUM") as pxs, \
                     tc.tile_pool(name="psp", bufs=2, space="PSUM") as psp, \
                     tc.tile_pool(name="pav", bufs=1, space="PSUM") as pav:
                    pools.update(pxs=pxs, psp=psp, pav=pav)
                    # sel-0 window: tiles (0,0..15); attn(0,half0) from slot 8
                    for i in range(NT):
                        sel = emit_sel_compute(0, i)
                        if i >= 8:
                            for jj in (2 * (i - 8), 2 * (i - 8) + 1):
                                emit_attn(0, 0, jj)
                        emit_sel_mask(0, i, *sel)
                    emit_attn_end(0, 0)
                    # sel-1 window: tiles (1,0..15); attn(0,half1) slots 0-7,
                    # attn(1,half0) slots 8-15
                    for i in range(NT):
                        sel = emit_sel_compute(1, i)
                        if i < 8:
                            for jj in (2 * i, 2 * i + 1):
                                emit_attn(0, 1, jj)
                        else:
                            if i == 8:
                                emit_attn_end(0, 1)
                                emit_C(0)
                            for jj in (2 * (i - 8), 2 * (i - 8) + 1):
                                emit_attn(1, 0, jj)
                        emit_sel_mask(1, i, *sel)
                    emit_attn_end(1, 0)

                # ---- tail: attn(1,half1) on fresh, deeper PSUM pools ----
                with tc.tile_pool(name="ptsp", bufs=4, space="PSUM") as tsp, \
                     tc.tile_pool(name="ptav", bufs=1, space="PSUM") as tav:
                    pools.update(psp=tsp, pav=tav)
                    for j in range(NT):
                        emit_attn(1, 1, j, mul_dve=True)
                    emit_attn_end(1, 1)
                    emit_C(1)

            # ================= out_proj =================
            with tc.tile_pool(name="po", bufs=2) as po, \
                 tc.tile_pool(name="pop", bufs=2, space="PSUM") as pop:
                for i in range(NT):
                    op = pop.tile([128, D], dt.float32, tag="op")
                    for n in range(2):
                        nc.tensor.matmul(op[:, 512 * n:512 * n + 512],
                                         ATcatT[:, 128 * i:128 * i + 128],
                                         wo[:, 512 * n:512 * n + 512])
                    ob = po.tile([128, D], dt.float32, tag="ob")
                    nc.scalar.copy(out=ob[:], in_=op[:])
                    nc.sync.dma_start(out=out_part[128 * i:128 * i + 128, :], in_=ob[:])

    _split_excess_waits(nc, limit=1)
    return nc


def _prep_inputs(hidden_states, Wq, Wk, Wv, Wo, idx_wq, idx_wk):
    hs = np.asarray(hidden_states[0], np.float32)          # [T, D]
    hsT = np.ascontiguousarray(hs.T)                       # [D, T]
    hsT_hi = hsT.astype(BF16)
    hsT_lo = (hsT - hsT_hi.astype(np.float32)).astype(BF16)
    maps = []
    for c in range(NCORES):
        h0, h1 = 2 * c, 2 * c + 1
        Aq_parts, Ak_parts = [], []
        for hh in (h0, h1):
            Wq_h = Wq[64 * hh:64 * hh + 64, :].astype(np.float64)    # [64, D]
            Wk_h = Wk[64 * hh:64 * hh + 64, :].astype(np.float64)
            Aq_parts.append((Wq_h.T @ idx_wq[hh].astype(np.float64)).astype(np.float32))
            Ak_parts.append((Wk_h.T @ idx_wk[hh].astype(np.float64)).astype(np.float32))
        A_cat = np.concatenate(Aq_parts + Ak_parts, axis=1)  # [D, 128]
        A_hi = A_cat.astype(BF16)
        A_lo = (A_cat - A_hi.astype(np.float32)).astype(BF16)

        def qk_chain(hh):
            Wq_h = Wq[64 * hh:64 * hh + 64, :]
            Wk_h = Wk[64 * hh:64 * hh + 64, :]
            return np.concatenate(
                [(Wq_h.T / np.sqrt(DH)).astype(BF16), Wk_h.T.astype(BF16)], axis=1)

        Wv_c = np.concatenate(
            [Wv[64 * h0:64 * h0 + 64, :].T, Wv[64 * h1:64 * h1 + 64, :].T],
            axis=1).astype(BF16)                           # [D, 128]
        WoT_c = np.ascontiguousarray(Wo[:, 64 * h0:64 * h0 + 128].T).astype(BF16)

        maps.append({
            "hsT_hi": hsT_hi,
            "hsT_lo": hsT_lo,
            "A_hi": A_hi,
            "A_lo": A_lo,
            "Wqk_h0": qk_chain(h0),
            "Wqk_h1": qk_chain(h1),
            "Wv_cat": Wv_c,
            "WoT_cat": WoT_c,
        })
    return maps


def kernel(hidden_states, Wq, Wk, Wv, Wo, idx_wq, idx_wk):
    from concourse.bass_utils import run_bass_kernel_spmd

    if "nc" not in _COMPILED:
        _COMPILED["nc"] = _build_module()
    nc = _COMPILED["nc"]

    in_maps = _prep_inputs(np.asarray(hidden_states), np.asarray(Wq),
                           np.asarray(Wk), np.asarray(Wv), np.asarray(Wo),
                           np.asarray(idx_wq), np.asarray(idx_wk))
    res = run_bass_kernel_spmd(nc, in_maps, core_ids=list(range(NCORES)))
    out = np.zeros((T, D), np.float32)
    for c in range(NCORES):
        out += np.asarray(res.results[c]["out_part"], np.float32)
    return out.reshape(B, T, D)


# revision 17
# speedup vs baseline: 1.0967x; 1.0967x over previous
"""DeepSeek sparse attention on 8 Trainium2 NeuronCores.

Head-sharded (2 heads/core). v2 schedule — single Act table set
({Exp, Copy, Sign} all live in exp_and_others -> zero table swaps),
selection/attention software-pipelined:

  - P0: indexer projection I = hs @ A via bf16 hi/lo 3-pass matmul; Q/K/V
    projections bf16, stationary-grouped loops.
  - selection per (h,i): X = iq@ik^T via 96-row hi/lo-stacked bf16 matmul;
    top-32 threshold per query via 16-subset DVE MAX8 + 4 rounds
    max8/match_replace; mask m = Sign(X - t32 + eps) in {-1,+1} on the
    Scalar engine, DMA-transposed to [s,q].
  - attention in [s,q]: e = exp(S^T) (Scalar); w' = e*m (GpSimd TT);
    AV accumulates BOTH e and w' into one PSUM group so av equals
    2*(masked AV) — the ones-row normalization absorbs the factor 2.
    The w'-AV matmuls are emitted one step late so the PE queue never
    head-of-line blocks on the GpSimd multiply.
  - schedule: selection is the DVE-paced spine; attention steps fill
    PE/Act/GpSimd under it: attn(0,half0) under sel-0 slots 8-15,
    attn(0,half1)+attn(1,half0) under sel-1, attn(1,half1) as the tail.
  - out_proj partial per core; host sums the 8 partials.
"""
import sys

sys.path.insert(0, '/opt/trn_rl_repo')
sys.path.insert(0, '/opt/pypackages')

import numpy as np
import ml_dtypes

BF16 = ml_dtypes.bfloat16

B, T, D = 1, 2048, 1024
H, DH, DI, KSEL = 16, 64, 32, 32
NCORES = 8
HPC = H // NCORES
NT = T // 128               # 16 query/key tiles
NK = D // 128               # 8 contraction chunks


MARGIN = 1e-5               # inclusion margin on the threshold

_COMPILED = {}


def _install_drain_patch():
    import concourse.mybir as mybir
    from concourse.tile import TileContext
    from concourse.vector_clock import ScopedClock

    if getattr(TileContext, "_dsa_patched", False):
        return

    def _patched(self, tick_clock, wait_clock):
        nc = self.nc
        drain_inst = nc.sync.drain()
        wait_clock.add_sem_waits(
            drain_inst.ins, ScopedClock({None: tick_clock.global_clock})
        )
        si = drain_inst.ins.sync_info
        waits = list(si.on_wait or []) if si is not None else []
        if len(waits) > 1:
            drain_inst.ins.sync_info = mybir.SyncInfo(
                on_wait=waits[:1], on_update=list(si.on_update or [])
            )
            for i in range(1, len(waits)):
                extra = nc.sync.drain()
                extra.ins.sync_info = mybir.SyncInfo(
                    on_wait=waits[i:i + 1], on_update=[]
                )
        nc.all_engine_barrier()
        assert self.sems is not None
        popped = nc._tile_sem_poison_stack.pop()
        assert popped is self._sem_poison
        nc.clear_and_free_semaphores(list(self.sems.allocated().values()))
        nc.all_engine_barrier()

    TileContext._drain_and_barrier = _patched
    TileContext._dsa_patched = True


def _split_excess_waits(nc, limit=1):
    """walrus in this container rejects instructions with more sync waits
    than the ISA struct encodes; hoist excess waits onto standalone
    EventSemaphore instructions on the same engine, inserted just before."""
    import concourse.mybir as mybir

    n_new = 0
    for bb in nc.main_func.blocks:
        insts = bb.instructions
        i = 0
        while i < len(insts):
            ins = insts[i]
            si = ins.sync_info
            waits = list(si.on_wait or []) if si is not None else []
            if len(waits) > limit:
                ins.sync_info = mybir.SyncInfo(
                    on_wait=waits[:limit], on_update=list(si.on_update or []))
                pos = i
                for j in range(limit, len(waits), limit):
                    n_new += 1
                    w = mybir.InstEventSemaphore(
                        name=f"WSPLIT-{n_new}", ins=[], outs=[])
                    w.engine = ins.engine
                    w.sync_info = mybir.SyncInfo(
                        on_wait=waits[j:j + limit], on_update=[])
                    nc.register_instruction(w, overwrite=True)
                    insts.insert(pos, w)
                    pos += 1
                    i += 1
            i += 1
    return n_new


def _build_module():
    import concourse.bass as bass
    import concourse.mybir as mybir
    from concourse.tile import TileContext

    _install_drain_patch()
    dt = mybir.dt
    nc = bass.Bass()

    hsT_hi = nc.declare_dram_parameter("hsT_hi", [D, T], dt.bfloat16, isOutput=False)
    hsT_lo = nc.declare_dram_parameter("hsT_lo", [D, T], dt.bfloat16, isOutput=False)
    A_hi = nc.declare_dram_parameter("A_hi", [D, 128], dt.bfloat16, isOutput=False)
    A_lo = nc.declare_dram_parameter("A_lo", [D, 128], dt.bfloat16, isOutput=False)
    Wqk_h0 = nc.declare_dram_parameter("Wqk_h0", [D, 128], dt.bfloat16, isOutput=False)
    Wqk_h1 = nc.declare_dram_parameter("Wqk_h1", [D, 128], dt.bfloat16, isOutput=False)
    Wv_cat = nc.declare_dram_parameter("Wv_cat", [D, 128], dt.bfloat16, isOutput=False)
    WoT_cat = nc.declare_dram_parameter("WoT_cat", [128, D], dt.bfloat16, isOutput=False)
    out_part = nc.declare_dram_parameter("out_part", [T, D], dt.float32, isOutput=True)

    Sign = mybir.ActivationFunctionType.Sign
    Exp = mybir.ActivationFunctionType.Exp
    Copy = mybir.ActivationFunctionType.Copy
    MUL = mybir.AluOpType.mult
    SUB = mybir.AluOpType.subtract
    ADD = mybir.AluOpType.add

    with TileContext(nc) as tc:
        with tc.tile_pool(name="state", bufs=1) as st:
            IqSs = [st.tile([96, T], dt.bfloat16, tag=f"IqS{h}", name=f"IqS{h}")
                    for h in range(2)]
            IkSs = [st.tile([96, T], dt.bfloat16, tag=f"IkS{h}", name=f"IkS{h}")
                    for h in range(2)]
            QT = st.tile([128, T], dt.bfloat16, tag="QT")
            KT = st.tile([128, T], dt.bfloat16, tag="KT")
            VP = st.tile([128, NT, 2, 65], dt.bfloat16, tag="VP")
            ATcatT = st.tile([128, T], dt.bfloat16, tag="ATcatT")
            wo = st.tile([128, D], dt.bfloat16, tag="wo")
            nc.sync.dma_start(out=wo[:], in_=WoT_cat[:])

            # ================= P0: projections =================
            with tc.tile_pool(name="hsbp", bufs=1) as hp, \
                 tc.tile_pool(name="p0w", bufs=1) as p0w, \
                 tc.tile_pool(name="p0p", bufs=1, space="PSUM") as p0p:
                hsb = hp.tile([128, NK, T], dt.bfloat16, tag="hsb")
                hslo = hp.tile([128, NK, T], dt.bfloat16, tag="hslo")
                a_h = p0w.tile([128, NK, 128], dt.bfloat16, tag="a_h")
                a_l = p0w.tile([128, NK, 128], dt.bfloat16, tag="a_l")
                qk0_w = p0w.tile([128, NK, 128], dt.bfloat16, tag="qk0_w")
                qk1_w = p0w.tile([128, NK, 128], dt.bfloat16, tag="qk1_w")
                v_w = p0w.tile([128, NK, 128], dt.bfloat16, tag="v_w")
                nc.sync.dma_start(out=a_h[:], in_=A_hi[:].rearrange("(c p) m -> p c m", p=128))
                nc.sync.dma_start(out=a_l[:], in_=A_lo[:].rearrange("(c p) m -> p c m", p=128))
                nc.sync.dma_start(out=qk0_w[:], in_=Wqk_h0[:].rearrange("(c p) m -> p c m", p=128))
                nc.sync.dma_start(out=qk1_w[:], in_=Wqk_h1[:].rearrange("(c p) m -> p c m", p=128))
                nc.sync.dma_start(out=v_w[:], in_=Wv_cat[:].rearrange("(c p) m -> p c m", p=128))
                for k in range(NK):
                    nc.sync.dma_start(out=hsb[:, k, :], in_=hsT_hi[128 * k:128 * k + 128, :])
                    nc.sync.dma_start(out=hslo[:, k, :], in_=hsT_lo[128 * k:128 * k + 128, :])

                ip = p0p.tile([128, T], dt.float32, tag="ip")
                qp = p0p.tile([128, T], dt.float32, tag="qp")
                # pass A: I (hi/lo 3-pass) + QK_h0, LDW-grouped per stationary
                for k in range(NK):
                    for n in range(4):
                        sl = slice(512 * n, 512 * n + 512)
                        nc.tensor.matmul(ip[:, sl], a_h[:, k, :], hsb[:, k, sl],
                                         start=(k == 0), stop=False)
                    for n in range(4):
                        sl = slice(512 * n, 512 * n + 512)
                        nc.tensor.matmul(ip[:, sl], a_h[:, k, :], hslo[:, k, sl],
                                         start=False, stop=False)
                    for n in range(4):
                        sl = slice(512 * n, 512 * n + 512)
                        nc.tensor.matmul(ip[:, sl], a_l[:, k, :], hsb[:, k, sl],
                                         start=False, stop=(k == NK - 1))
                    for n in range(4):
                        sl = slice(512 * n, 512 * n + 512)
                        nc.tensor.matmul(qp[:, sl], qk0_w[:, k, :], hsb[:, k, sl],
                                         start=(k == 0), stop=(k == NK - 1))
                # evac indexer projections: contraction-stacked hi/lo
                for h in range(2):
                    iqr = ip[32 * h:32 * h + 32, :]
                    ikr = ip[64 + 32 * h:64 + 32 * h + 32, :]
                    IqS, IkS = IqSs[h], IkSs[h]
                    nc.scalar.copy(out=IqS[0:32, :], in_=iqr)
                    nc.vector.tensor_copy(IqS[32:64, :], IqS[0:32, :])
                    nc.vector.tensor_tensor(out=IqS[64:96, :], in0=iqr,
                                            in1=IqS[0:32, :], op=SUB)
                    nc.scalar.copy(out=IkS[0:32, :], in_=ikr)
                    nc.vector.tensor_tensor(out=IkS[32:64, :], in0=ikr,
                                            in1=IkS[0:32, :], op=SUB)
                    nc.vector.tensor_copy(IkS[64:96, :], IkS[0:32, :])
                nc.scalar.copy(out=QT[0:64, :], in_=qp[0:64, :])
                nc.scalar.copy(out=KT[0:64, :], in_=qp[64:128, :])

                # pass B: QK_h1 + V
                qp1 = p0p.tile([128, T], dt.float32, tag="ip")  # reuse slot
                vp_ps = p0p.tile([128, T], dt.float32, tag="qp")
                for k in range(NK):
                    for n in range(4):
                        sl = slice(512 * n, 512 * n + 512)
                        nc.tensor.matmul(qp1[:, sl], qk1_w[:, k, :], hsb[:, k, sl],
                                         start=(k == 0), stop=(k == NK - 1))
                    for n in range(4):
                        sl = slice(512 * n, 512 * n + 512)
                        nc.tensor.matmul(vp_ps[:, sl], v_w[:, k, :], hsb[:, k, sl],
                                         start=(k == 0), stop=(k == NK - 1))
                nc.scalar.copy(out=QT[64:128, :], in_=qp1[0:64, :])
                nc.scalar.copy(out=KT[64:128, :], in_=qp1[64:128, :])
                # V: rows 0:64 = V_h0^T [dh, s], 64:128 = V_h1^T; DMA-transpose
                vt_b = p0w.tile([128, T], dt.bfloat16, tag="vt_b")
                nc.scalar.copy(out=vt_b[:], in_=vp_ps[:])
                vq = p0w.tile([128, NT, 128], dt.bfloat16, tag="vq")
                nc.sync.dma_start_transpose(out=vq[:], in_=vt_b[:])
                for j in range(NT):
                    for h in range(2):
                        nc.scalar.copy(out=VP[:, j, h, 0:64],
                                       in_=vq[:, j, 64 * h:64 * h + 64])
                nc.vector.memset(VP[:, :, :, 64:65], 1.0)

            # ================= selection + attention pipeline =================
            with tc.tile_pool(name="mtp", bufs=2) as mtp, \
                 tc.tile_pool(name="pa", bufs=2) as pa, \
                 tc.tile_pool(name="pms", bufs=2) as pms, \
                 tc.tile_pool(name="pe_", bufs=2) as pe_, \
                 tc.tile_pool(name="pc", bufs=1) as pc:
                pools = {}

                mts = [mtp.tile([128, NT, NT, 128], dt.bfloat16, tag="mt",
                                name=f"mt{h}") for h in range(2)]
                ats = [pc.tile([128, T], dt.bfloat16, tag=f"at{h}", name=f"at{h}")
                       for h in range(2)]

                def emit_sel_compute(h, i):
                    """X matmuls + candidate top-8s + rounds -> negt; returns
                    (xps, negt) for the deferred mask pass"""
                    cand = pa.tile([128, 128], dt.float32, tag="cand")
                    xps = []
                    for half in range(2):
                        xp = pools["pxs"].tile([128, 1024], dt.float32, tag="xps",
                                      name=f"xp{h}_{i}_{half}")
                        xps.append(xp)
                        for n in range(2):
                            sl = slice(512 * n, 512 * n + 512)
                            c0 = 1024 * half + 512 * n
                            qsl = slice(128 * i, 128 * i + 128)
                            nc.tensor.matmul(xp[:, sl], IqSs[h][:, qsl],
                                             IkSs[h][:, c0:c0 + 512])
                        v8 = xp[:].rearrange("p (s l) -> p l s", l=8)
                        for j in range(8):
                            nc.vector.max(out=cand[:, 64 * half + 8 * j:64 * half + 8 * j + 8],
                                          in_=v8[:, j, :])
                    mx = pa.tile([128, 8], dt.float32, tag="mx")
                    for r in range(4):
                        nc.vector.max(out=mx[:], in_=cand[:])
                        if r < 3:
                            nc.vector.match_replace(out=cand[:], in_to_replace=mx[:],
                                                    in_values=cand[:], imm_value=-1e30)
                    # negt = -t32 + margin; m = Sign(X + negt) in {-1,+1}
                    # (Sign lives in exp_and_others with Exp/Copy: no table swap)
                    negt = pa.tile([128, 1], dt.float32, tag="negt")
                    nc.vector.tensor_scalar(negt[:], mx[:, 7:8], -1.0,
                                            scalar2=MARGIN, op0=MUL, op1=ADD)
                    return xps, negt

                def emit_sel_mask(h, i, xps, negt):
                    """Sign masks + transpose. Emitted AFTER the slot's attn
                    exps so the Act queue never head-of-line blocks on negt."""
                    mt = mts[h]
                    ms = pms.tile([128, T], dt.bfloat16, tag="ms", name=f"ms{h}_{i}")
                    for half in range(2):
                        nc.scalar.activation(
                            out=ms[:, 1024 * half:1024 * half + 1024],
                            in_=xps[half][:], func=Sign, bias=negt[:])
                        nc.tensor.ldweights(weights=ms[:, 1024 * half:1024 * half + 128])
                    nc.sync.dma_start_transpose(out=mt[:, :, i, :], in_=ms[:])

                # per-stream state: current av tile + the lagged w'-AV emission
                avst = {"av": None, "pending": None}

                def flush_wav(last=False):
                    """emit the lagged w'-AV matmuls for the previous step"""
                    p = avst["pending"]
                    if p is None:
                        return
                    avst["pending"] = None
                    av, wt, h, j = p
                    for n in range(2):
                        sl = slice(512 * n, 512 * n + 512)
                        nc.tensor.matmul(av[:, sl], VP[:, j, h, :], wt[:, sl],
                                         start=False, stop=last)

                def emit_attn(h, half, j, mul_dve=False):
                    """attention step: sp = K_j^T Q_half; e = exp(sp);
                    w' = e * sign-mask (GpSimd TT, or DVE TT in the tail where
                    DVE is idle); av += VP@(e) and += VP@(w') so av equals
                    2*(masked AV) — the ones-row normalization absorbs it.
                    The w'-AV matmuls are emitted one step late so the PE queue
                    never head-of-line blocks on the mask multiply. A dummy
                    LDWEIGHTS dep'd on e fires mid-slot to keep the PE HAM
                    activity monitor fed (cold PE runs at half clock)."""
                    mt = mts[h]
                    sp = pools["psp"].tile([128, 1024], dt.float32, tag="sp",
                                           name=f"sp{h}_{half}_{j}")
                    for n in range(2):
                        sl = slice(512 * n, 512 * n + 512)
                        c0 = 1024 * half + 512 * n
                        nc.tensor.matmul(sp[:, sl],
                                         KT[64 * h:64 * h + 64, 128 * j:128 * j + 128],
                                         QT[64 * h:64 * h + 64, c0:c0 + 512])
                    e = pe_.tile([128, 1024], dt.bfloat16, tag="e", name="e")
                    nc.scalar.activation(out=e[:], in_=sp[:], func=Exp)
                    msl = mt[:, j, 8 * half:8 * half + 8, :].rearrange("p a b -> p (a b)")
                    wt = pe_.tile([128, 1024], dt.bfloat16, tag="w", name="w")
                    if mul_dve:
                        nc.vector.tensor_tensor(out=wt[:], in0=e[:], in1=msl, op=MUL)
                    else:
                        nc.gpsimd.tensor_tensor(out=wt[:], in0=e[:], in1=msl, op=MUL)
                    if j == 0:
                        avst["av"] = pools["pav"].tile([65, 1024], dt.float32, tag="av",
                                                       name=f"av{h}_{half}")
                    av = avst["av"]
                    for n in range(2):
                        sl = slice(512 * n, 512 * n + 512)
                        nc.tensor.matmul(av[:, sl], VP[:, j, h, :], e[:, sl],
                                         start=(j == 0), stop=False)
                    flush_wav()
                    avst["pending"] = (av, wt, h, j)
                    nc.tensor.ldweights(weights=e[:, 0:128])

                def emit_attn_end(h, half):
                    flush_wav(last=True)
                    nc.scalar.copy(out=ats[h][0:65, 1024 * half:1024 * half + 1024],
                                   in_=avst["av"][:])

                def emit_C(h):
                    """normalize + build transposed attn rows of ATcatT"""
                    at = ats[h]
                    atq = pc.tile([128, NT, 128], dt.bfloat16, tag="atq")
                    nc.sync.dma_start_transpose(out=atq[:], in_=at[:])
                    scrall = pc.tile([128, NT, 128], dt.bfloat16, tag="scrall")
                    rds = pa.tile([128, NT], dt.float32, tag="rds")
                    nc.vector.reciprocal(
                        rds[:], atq[:, :, 64:65].rearrange("p a b -> p (a b)"))
                    for i in range(NT):
                        nc.scalar.activation(out=scrall[:, i, 0:64],
                                             in_=atq[:, i, 0:64], func=Copy,
                                             scale=rds[:, i:i + 1])
                    tmpT = pc.tile([128, NT, 128], dt.bfloat16, tag="tmpT")
                    nc.sync.dma_start_transpose(out=tmpT[:], in_=scrall[:])
                    nc.vector.tensor_copy(ATcatT[64 * h:64 * h + 64, :],
                                          tmpT[0:64, :, :].rearrange("p a b -> p (a b)"))

                # ---- issue schedule ----
                with tc.tile_pool(name="pxs", bufs=2, space="PSUM") as pxs, \
                     tc.tile_pool(name="psp", bufs=1, space="PSUM") as psp, \
                     tc.tile_pool(name="pav", bufs=1, space="PSUM") as pav:
                    pools.update(pxs=pxs, psp=psp, pav=pav)
                    # sel-0 window: tiles (0,0..15); attn(0,half0) from slot 8
                    for i in range(NT):
                        sel = emit_sel_compute(0, i)
                        if i >= 8:
                            for jj in (2 * (i - 8), 2 * (i - 8) + 1):
                                emit_attn(0, 0, jj)
                        emit_sel_mask(0, i, *sel)
                    emit_attn_end(0, 0)
                    # sel-1 window: tiles (1,0..15); attn(0,half1) slots 0-7,
                    # attn(1,half0) slots 8-15
                    for i in range(NT):
                        sel = emit_sel_compute(1, i)
                        if i < 8:
                            for jj in (2 * i, 2 * i + 1):
                                emit_attn(0, 1, jj)
                        else:
                            if i == 8:
                                emit_attn_end(0, 1)
                                emit_C(0)
                            for jj in (2 * (i - 8), 2 * (i - 8) + 1):
                                emit_attn(1, 0, jj)
                        emit_sel_mask(1, i, *sel)
                    emit_attn_end(1, 0)

                # ---- tail: attn(1,half1) on fresh, deeper PSUM pools ----
                with tc.tile_pool(name="ptsp", bufs=2, space="PSUM") as tsp, \
                     tc.tile_pool(name="ptav", bufs=1, space="PSUM") as tav:
                    pools.update(psp=tsp, pav=tav)
                    for j in range(NT):
                        emit_attn(1, 1, j, mul_dve=True)
                    emit_attn_end(1, 1)
                    emit_C(1)

            # ================= out_proj =================
            with tc.tile_pool(name="po", bufs=2) as po, \
                 tc.tile_pool(name="pop", bufs=2, space="PSUM") as pop:
                for i in range(NT):
                    op = pop.tile([128, D], dt.float32, tag="op")
                    for n in range(2):
                        nc.tensor.matmul(op[:, 512 * n:512 * n + 512],
                                         ATcatT[:, 128 * i:128 * i + 128],
                                         wo[:, 512 * n:512 * n + 512])
                    ob = po.tile([128, D], dt.float32, tag="ob")
                    nc.scalar.copy(out=ob[:], in_=op[:])
                    nc.sync.dma_start(out=out_part[128 * i:128 * i + 128, :], in_=ob[:])

    _split_excess_waits(nc, limit=1)
    return nc


def _prep_inputs(hidden_states, Wq, Wk, Wv, Wo, idx_wq, idx_wk):
    hs = np.asarray(hidden_states[0], np.float32)          # [T, D]
    hsT = np.ascontiguousarray(hs.T)                       # [D, T]
    hsT_hi = hsT.astype(BF16)
    hsT_lo = (hsT - hsT_hi.astype(np.float32)).astype(BF16)
    maps = []
    for c in range(NCORES):
        h0, h1 = 2 * c, 2 * c + 1
        Aq_parts, Ak_parts = [], []
        for hh in (h0, h1):
            Wq_h = Wq[64 * hh:64 * hh + 64, :].astype(np.float64)    # [64, D]
            Wk_h = Wk[64 * hh:64 * hh + 64, :].astype(np.float64)
            Aq_parts.append((Wq_h.T @ idx_wq[hh].astype(np.float64)).astype(np.float32))
            Ak_parts.append((Wk_h.T @ idx_wk[hh].astype(np.float64)).astype(np.float32))
        A_cat = np.concatenate(Aq_parts + Ak_parts, axis=1)  # [D, 128]
        A_hi = A_cat.astype(BF16)
        A_lo = (A_cat - A_hi.astype(np.float32)).astype(BF16)

        def qk_chain(hh):
            Wq_h = Wq[64 * hh:64 * hh + 64, :]
            Wk_h = Wk[64 * hh:64 * hh + 64, :]
            return np.concatenate(
                [(Wq_h.T / np.sqrt(DH)).astype(BF16), Wk_h.T.astype(BF16)], axis=1)

        Wv_c = np.concatenate(
            [Wv[64 * h0:64 * h0 + 64, :].T, Wv[64 * h1:64 * h1 + 64, :].T],
            axis=1).astype(BF16)                           # [D, 128]
        WoT_c = np.ascontiguousarray(Wo[:, 64 * h0:64 * h0 + 128].T).astype(BF16)

        maps.append({
            "hsT_hi": hsT_hi,
            "hsT_lo": hsT_lo,
            "A_hi": A_hi,
            "A_lo": A_lo,
            "Wqk_h0": qk_chain(h0),
            "Wqk_h1": qk_chain(h1),
            "Wv_cat": Wv_c,
            "WoT_cat": WoT_c,
        })
    return maps


def kernel(hidden_states, Wq, Wk, Wv, Wo, idx_wq, idx_wk):
    from concourse.bass_utils import run_bass_kernel_spmd

    if "nc" not in _COMPILED:
        _COMPILED["nc"] = _build_module()
    nc = _COMPILED["nc"]

    in_maps = _prep_inputs(np.asarray(hidden_states), np.asarray(Wq),
                           np.asarray(Wk), np.asarray(Wv), np.asarray(Wo),
                           np.asarray(idx_wq), np.asarray(idx_wk))
    res = run_bass_kernel_spmd(nc, in_maps, core_ids=list(range(NCORES)))
    out = np.zeros((T, D), np.float32)
    for c in range(NCORES):
        out += np.asarray(res.results[c]["out_part"], np.float32)
    return out.reshape(B, T, D)


# revision 18
# speedup vs baseline: 1.1361x; 1.0359x over previous
"""DeepSeek sparse attention on 8 Trainium2 NeuronCores.

Head-sharded (2 heads/core). v2 schedule — single Act table set
({Exp, Copy, Sign} all live in exp_and_others -> zero table swaps),
selection/attention software-pipelined:

  - P0: indexer projection I = hs @ A via bf16 hi/lo 3-pass matmul; Q/K/V
    projections bf16, stationary-grouped loops.
  - selection per (h,i): X = iq@ik^T via 96-row hi/lo-stacked bf16 matmul;
    top-32 threshold per query via 16-subset DVE MAX8 + 4 rounds
    max8/match_replace; mask m = Sign(X - t32 + eps) in {-1,+1} on the
    Scalar engine, DMA-transposed to [s,q].
  - attention in [s,q]: e = exp(S^T) (Scalar); w' = e*m (GpSimd TT);
    AV accumulates BOTH e and w' into one PSUM group so av equals
    2*(masked AV) — the ones-row normalization absorbs the factor 2.
    The w'-AV matmuls are emitted one step late so the PE queue never
    head-of-line blocks on the GpSimd multiply.
  - schedule: selection is the DVE-paced spine; attention steps fill
    PE/Act/GpSimd under it: attn(0,half0) under sel-0 slots 8-15,
    attn(0,half1)+attn(1,half0) under sel-1, attn(1,half1) as the tail.
  - out_proj partial per core; host sums the 8 partials.
"""
import sys

sys.path.insert(0, '/opt/trn_rl_repo')
sys.path.insert(0, '/opt/pypackages')

import numpy as np
import ml_dtypes

BF16 = ml_dtypes.bfloat16

B, T, D = 1, 2048, 1024
H, DH, DI, KSEL = 16, 64, 32, 32
NCORES = 8
HPC = H // NCORES
NT = T // 128               # 16 query/key tiles
NK = D // 128               # 8 contraction chunks


MARGIN = 1e-5               # inclusion margin on the threshold

_COMPILED = {}


def _install_ldwopt_patch():
    """bass's walrus invocation pins --enable-ldw-opt=false; this kernel's
    matmul streams reuse the same stationary for consecutive matmuls
    (S^T pairs, AV pairs, P0 groups), so the LDW optimizer saves a large
    fraction of the ~860 LDWEIGHTS on the PE queue. Flip it on."""
    import concourse.bass_utils as bu

    if getattr(bu, "_dsa_ldwopt_patched", False):
        return
    orig = bu.get_walrus_args

    def patched(*args, **kwargs):
        out = orig(*args, **kwargs)
        return [a.replace("--enable-ldw-opt=false", "--enable-ldw-opt=true")
                if isinstance(a, str) else a for a in out]

    bu.get_walrus_args = patched
    bu._dsa_ldwopt_patched = True


def _install_drain_patch():
    import concourse.mybir as mybir
    from concourse.tile import TileContext
    from concourse.vector_clock import ScopedClock

    if getattr(TileContext, "_dsa_patched", False):
        return

    def _patched(self, tick_clock, wait_clock):
        nc = self.nc
        drain_inst = nc.sync.drain()
        wait_clock.add_sem_waits(
            drain_inst.ins, ScopedClock({None: tick_clock.global_clock})
        )
        si = drain_inst.ins.sync_info
        waits = list(si.on_wait or []) if si is not None else []
        if len(waits) > 1:
            drain_inst.ins.sync_info = mybir.SyncInfo(
                on_wait=waits[:1], on_update=list(si.on_update or [])
            )
            for i in range(1, len(waits)):
                extra = nc.sync.drain()
                extra.ins.sync_info = mybir.SyncInfo(
                    on_wait=waits[i:i + 1], on_update=[]
                )
        nc.all_engine_barrier()
        assert self.sems is not None
        popped = nc._tile_sem_poison_stack.pop()
        assert popped is self._sem_poison
        nc.clear_and_free_semaphores(list(self.sems.allocated().values()))
        nc.all_engine_barrier()

    TileContext._drain_and_barrier = _patched
    TileContext._dsa_patched = True


def _split_excess_waits(nc, limit=1):
    """walrus in this container rejects instructions with more sync waits
    than the ISA struct encodes; hoist excess waits onto standalone
    EventSemaphore instructions on the same engine, inserted just before."""
    import concourse.mybir as mybir

    n_new = 0
    for bb in nc.main_func.blocks:
        insts = bb.instructions
        i = 0
        while i < len(insts):
            ins = insts[i]
            si = ins.sync_info
            waits = list(si.on_wait or []) if si is not None else []
            if len(waits) > limit:
                ins.sync_info = mybir.SyncInfo(
                    on_wait=waits[:limit], on_update=list(si.on_update or []))
                pos = i
                for j in range(limit, len(waits), limit):
                    n_new += 1
                    w = mybir.InstEventSemaphore(
                        name=f"WSPLIT-{n_new}", ins=[], outs=[])
                    w.engine = ins.engine
                    w.sync_info = mybir.SyncInfo(
                        on_wait=waits[j:j + limit], on_update=[])
                    nc.register_instruction(w, overwrite=True)
                    insts.insert(pos, w)
                    pos += 1
                    i += 1
            i += 1
    return n_new


def _build_module():
    import concourse.bass as bass
    import concourse.mybir as mybir
    from concourse.tile import TileContext

    _install_drain_patch()
    _install_ldwopt_patch()
    dt = mybir.dt
    nc = bass.Bass()

    hsT_hi = nc.declare_dram_parameter("hsT_hi", [D, T], dt.bfloat16, isOutput=False)
    hsT_lo = nc.declare_dram_parameter("hsT_lo", [D, T], dt.bfloat16, isOutput=False)
    A_hi = nc.declare_dram_parameter("A_hi", [D, 128], dt.bfloat16, isOutput=False)
    A_lo = nc.declare_dram_parameter("A_lo", [D, 128], dt.bfloat16, isOutput=False)
    Wqk_h0 = nc.declare_dram_parameter("Wqk_h0", [D, 128], dt.bfloat16, isOutput=False)
    Wqk_h1 = nc.declare_dram_parameter("Wqk_h1", [D, 128], dt.bfloat16, isOutput=False)
    Wv_cat = nc.declare_dram_parameter("Wv_cat", [D, 128], dt.bfloat16, isOutput=False)
    WoT_cat = nc.declare_dram_parameter("WoT_cat", [128, D], dt.bfloat16, isOutput=False)
    out_part = nc.declare_dram_parameter("out_part", [T, D], dt.float32, isOutput=True)

    Sign = mybir.ActivationFunctionType.Sign
    Exp = mybir.ActivationFunctionType.Exp
    Copy = mybir.ActivationFunctionType.Copy
    MUL = mybir.AluOpType.mult
    SUB = mybir.AluOpType.subtract
    ADD = mybir.AluOpType.add

    with TileContext(nc) as tc:
        with tc.tile_pool(name="state", bufs=1) as st:
            IqSs = [st.tile([96, T], dt.bfloat16, tag=f"IqS{h}", name=f"IqS{h}")
                    for h in range(2)]
            IkSs = [st.tile([96, T], dt.bfloat16, tag=f"IkS{h}", name=f"IkS{h}")
                    for h in range(2)]
            QT = st.tile([128, T], dt.bfloat16, tag="QT")
            KT = st.tile([128, T], dt.bfloat16, tag="KT")
            VP = st.tile([128, NT, 2, 65], dt.bfloat16, tag="VP")
            ATcatT = st.tile([128, T], dt.bfloat16, tag="ATcatT")
            wo = st.tile([128, D], dt.bfloat16, tag="wo")
            nc.sync.dma_start(out=wo[:], in_=WoT_cat[:])

            # ================= P0: projections =================
            with tc.tile_pool(name="hsbp", bufs=1) as hp, \
                 tc.tile_pool(name="p0w", bufs=1) as p0w, \
                 tc.tile_pool(name="p0p", bufs=1, space="PSUM") as p0p:
                hsb = hp.tile([128, NK, T], dt.bfloat16, tag="hsb")
                hslo = hp.tile([128, NK, T], dt.bfloat16, tag="hslo")
                a_h = p0w.tile([128, NK, 128], dt.bfloat16, tag="a_h")
                a_l = p0w.tile([128, NK, 128], dt.bfloat16, tag="a_l")
                qk0_w = p0w.tile([128, NK, 128], dt.bfloat16, tag="qk0_w")
                qk1_w = p0w.tile([128, NK, 128], dt.bfloat16, tag="qk1_w")
                v_w = p0w.tile([128, NK, 128], dt.bfloat16, tag="v_w")
                nc.sync.dma_start(out=a_h[:], in_=A_hi[:].rearrange("(c p) m -> p c m", p=128))
                nc.sync.dma_start(out=a_l[:], in_=A_lo[:].rearrange("(c p) m -> p c m", p=128))
                nc.sync.dma_start(out=qk0_w[:], in_=Wqk_h0[:].rearrange("(c p) m -> p c m", p=128))
                nc.sync.dma_start(out=qk1_w[:], in_=Wqk_h1[:].rearrange("(c p) m -> p c m", p=128))
                nc.sync.dma_start(out=v_w[:], in_=Wv_cat[:].rearrange("(c p) m -> p c m", p=128))
                for k in range(NK):
                    nc.sync.dma_start(out=hsb[:, k, :], in_=hsT_hi[128 * k:128 * k + 128, :])
                    nc.sync.dma_start(out=hslo[:, k, :], in_=hsT_lo[128 * k:128 * k + 128, :])

                ip = p0p.tile([128, T], dt.float32, tag="ip")
                qp = p0p.tile([128, T], dt.float32, tag="qp")
                # pass A: I (hi/lo 3-pass) + QK_h0, LDW-grouped per stationary
                for k in range(NK):
                    for n in range(4):
                        sl = slice(512 * n, 512 * n + 512)
                        nc.tensor.matmul(ip[:, sl], a_h[:, k, :], hsb[:, k, sl],
                                         start=(k == 0), stop=False)
                    for n in range(4):
                        sl = slice(512 * n, 512 * n + 512)
                        nc.tensor.matmul(ip[:, sl], a_h[:, k, :], hslo[:, k, sl],
                                         start=False, stop=False)
                    for n in range(4):
                        sl = slice(512 * n, 512 * n + 512)
                        nc.tensor.matmul(ip[:, sl], a_l[:, k, :], hsb[:, k, sl],
                                         start=False, stop=(k == NK - 1))
                    for n in range(4):
                        sl = slice(512 * n, 512 * n + 512)
                        nc.tensor.matmul(qp[:, sl], qk0_w[:, k, :], hsb[:, k, sl],
                                         start=(k == 0), stop=(k == NK - 1))
                # evac indexer projections: contraction-stacked hi/lo
                for h in range(2):
                    iqr = ip[32 * h:32 * h + 32, :]
                    ikr = ip[64 + 32 * h:64 + 32 * h + 32, :]
                    IqS, IkS = IqSs[h], IkSs[h]
                    nc.scalar.copy(out=IqS[0:32, :], in_=iqr)
                    nc.vector.tensor_copy(IqS[32:64, :], IqS[0:32, :])
                    nc.vector.tensor_tensor(out=IqS[64:96, :], in0=iqr,
                                            in1=IqS[0:32, :], op=SUB)
                    nc.scalar.copy(out=IkS[0:32, :], in_=ikr)
                    nc.vector.tensor_tensor(out=IkS[32:64, :], in0=ikr,
                                            in1=IkS[0:32, :], op=SUB)
                    nc.vector.tensor_copy(IkS[64:96, :], IkS[0:32, :])
                nc.scalar.copy(out=QT[0:64, :], in_=qp[0:64, :])
                nc.scalar.copy(out=KT[0:64, :], in_=qp[64:128, :])

                # pass B: QK_h1 + V
                qp1 = p0p.tile([128, T], dt.float32, tag="ip")  # reuse slot
                vp_ps = p0p.tile([128, T], dt.float32, tag="qp")
                for k in range(NK):
                    for n in range(4):
                        sl = slice(512 * n, 512 * n + 512)
                        nc.tensor.matmul(qp1[:, sl], qk1_w[:, k, :], hsb[:, k, sl],
                                         start=(k == 0), stop=(k == NK - 1))
                    for n in range(4):
                        sl = slice(512 * n, 512 * n + 512)
                        nc.tensor.matmul(vp_ps[:, sl], v_w[:, k, :], hsb[:, k, sl],
                                         start=(k == 0), stop=(k == NK - 1))
                nc.scalar.copy(out=QT[64:128, :], in_=qp1[0:64, :])
                nc.scalar.copy(out=KT[64:128, :], in_=qp1[64:128, :])
                # V: rows 0:64 = V_h0^T [dh, s], 64:128 = V_h1^T; DMA-transpose
                vt_b = p0w.tile([128, T], dt.bfloat16, tag="vt_b")
                nc.scalar.copy(out=vt_b[:], in_=vp_ps[:])
                vq = p0w.tile([128, NT, 128], dt.bfloat16, tag="vq")
                nc.sync.dma_start_transpose(out=vq[:], in_=vt_b[:])
                for j in range(NT):
                    for h in range(2):
                        nc.scalar.copy(out=VP[:, j, h, 0:64],
                                       in_=vq[:, j, 64 * h:64 * h + 64])
                nc.vector.memset(VP[:, :, :, 64:65], 1.0)

            # ================= selection + attention pipeline =================
            with tc.tile_pool(name="mtp", bufs=2) as mtp, \
                 tc.tile_pool(name="pa", bufs=2) as pa, \
                 tc.tile_pool(name="pms", bufs=2) as pms, \
                 tc.tile_pool(name="pe_", bufs=2) as pe_, \
                 tc.tile_pool(name="pc", bufs=1) as pc:
                pools = {}

                mts = [mtp.tile([128, NT, NT, 128], dt.bfloat16, tag="mt",
                                name=f"mt{h}") for h in range(2)]
                ats = [pc.tile([128, T], dt.bfloat16, tag=f"at{h}", name=f"at{h}")
                       for h in range(2)]

                def emit_sel_compute(h, i):
                    """X matmuls + candidate top-8s + rounds -> negt; returns
                    (xps, negt) for the deferred mask pass"""
                    cand = pa.tile([128, 128], dt.float32, tag="cand")
                    xps = []
                    for half in range(2):
                        xp = pools["pxs"].tile([128, 1024], dt.float32, tag="xps",
                                      name=f"xp{h}_{i}_{half}")
                        xps.append(xp)
                        for n in range(2):
                            sl = slice(512 * n, 512 * n + 512)
                            c0 = 1024 * half + 512 * n
                            qsl = slice(128 * i, 128 * i + 128)
                            nc.tensor.matmul(xp[:, sl], IqSs[h][:, qsl],
                                             IkSs[h][:, c0:c0 + 512])
                        v8 = xp[:].rearrange("p (s l) -> p l s", l=8)
                        for j in range(8):
                            nc.vector.max(out=cand[:, 64 * half + 8 * j:64 * half + 8 * j + 8],
                                          in_=v8[:, j, :])
                    mx = pa.tile([128, 8], dt.float32, tag="mx")
                    for r in range(4):
                        nc.vector.max(out=mx[:], in_=cand[:])
                        if r < 3:
                            nc.vector.match_replace(out=cand[:], in_to_replace=mx[:],
                                                    in_values=cand[:], imm_value=-1e30)
                    # negt = -t32 + margin; m = Sign(X + negt) in {-1,+1}
                    # (Sign lives in exp_and_others with Exp/Copy: no table swap)
                    negt = pa.tile([128, 1], dt.float32, tag="negt")
                    nc.vector.tensor_scalar(negt[:], mx[:, 7:8], -1.0,
                                            scalar2=MARGIN, op0=MUL, op1=ADD)
                    return xps, negt

                def emit_sel_mask(h, i, xps, negt):
                    """Sign masks + transpose. Emitted AFTER the slot's attn
                    exps so the Act queue never head-of-line blocks on negt."""
                    mt = mts[h]
                    ms = pms.tile([128, T], dt.bfloat16, tag="ms", name=f"ms{h}_{i}")
                    for half in range(2):
                        nc.scalar.activation(
                            out=ms[:, 1024 * half:1024 * half + 1024],
                            in_=xps[half][:], func=Sign, bias=negt[:])
                    nc.sync.dma_start_transpose(out=mt[:, :, i, :], in_=ms[:])

                # per-stream state: current av tile + the lagged w'-AV emission
                avst = {"av": None, "pending": None}

                def flush_wav(last=False):
                    """emit the lagged w'-AV matmuls for the previous step"""
                    p = avst["pending"]
                    if p is None:
                        return
                    avst["pending"] = None
                    av, wt, h, j = p
                    for n in range(2):
                        sl = slice(512 * n, 512 * n + 512)
                        nc.tensor.matmul(av[:, sl], VP[:, j, h, :], wt[:, sl],
                                         start=False, stop=last)

                def emit_attn(h, half, j, mul_dve=False):
                    """attention step: sp = K_j^T Q_half; e = exp(sp);
                    w' = e * sign-mask (GpSimd TT, or DVE TT in the tail where
                    DVE is idle); av += VP@(e) and += VP@(w') so av equals
                    2*(masked AV) — the ones-row normalization absorbs it.
                    The w'-AV matmuls are emitted one step late so the PE queue
                    never head-of-line blocks on the mask multiply. A dummy
                    LDWEIGHTS dep'd on e fires mid-slot to keep the PE HAM
                    activity monitor fed (cold PE runs at half clock)."""
                    mt = mts[h]
                    sp = pools["psp"].tile([128, 1024], dt.float32, tag="sp",
                                           name=f"sp{h}_{half}_{j}")
                    for n in range(2):
                        sl = slice(512 * n, 512 * n + 512)
                        c0 = 1024 * half + 512 * n
                        nc.tensor.matmul(sp[:, sl],
                                         KT[64 * h:64 * h + 64, 128 * j:128 * j + 128],
                                         QT[64 * h:64 * h + 64, c0:c0 + 512])
                    e = pe_.tile([128, 1024], dt.bfloat16, tag="e", name="e")
                    nc.scalar.activation(out=e[:], in_=sp[:], func=Exp)
                    msl = mt[:, j, 8 * half:8 * half + 8, :].rearrange("p a b -> p (a b)")
                    wt = pe_.tile([128, 1024], dt.bfloat16, tag="w", name="w")
                    if mul_dve:
                        nc.vector.tensor_tensor(out=wt[:], in0=e[:], in1=msl, op=MUL)
                    else:
                        nc.gpsimd.tensor_tensor(out=wt[:], in0=e[:], in1=msl, op=MUL)
                    if j == 0:
                        avst["av"] = pools["pav"].tile([65, 1024], dt.float32, tag="av",
                                                       name=f"av{h}_{half}")
                    av = avst["av"]
                    for n in range(2):
                        sl = slice(512 * n, 512 * n + 512)
                        nc.tensor.matmul(av[:, sl], VP[:, j, h, :], e[:, sl],
                                         start=(j == 0), stop=False)
                    flush_wav()
                    avst["pending"] = (av, wt, h, j)

                def emit_attn_end(h, half):
                    flush_wav(last=True)
                    nc.scalar.copy(out=ats[h][0:65, 1024 * half:1024 * half + 1024],
                                   in_=avst["av"][:])

                def emit_C(h):
                    """normalize + build transposed attn rows of ATcatT"""
                    at = ats[h]
                    atq = pc.tile([128, NT, 128], dt.bfloat16, tag="atq")
                    nc.sync.dma_start_transpose(out=atq[:], in_=at[:])
                    scrall = pc.tile([128, NT, 128], dt.bfloat16, tag="scrall")
                    rds = pa.tile([128, NT], dt.float32, tag="rds")
                    nc.vector.reciprocal(
                        rds[:], atq[:, :, 64:65].rearrange("p a b -> p (a b)"))
                    for i in range(NT):
                        nc.scalar.activation(out=scrall[:, i, 0:64],
                                             in_=atq[:, i, 0:64], func=Copy,
                                             scale=rds[:, i:i + 1])
                    tmpT = pc.tile([128, NT, 128], dt.bfloat16, tag="tmpT")
                    nc.sync.dma_start_transpose(out=tmpT[:], in_=scrall[:])
                    nc.vector.tensor_copy(ATcatT[64 * h:64 * h + 64, :],
                                          tmpT[0:64, :, :].rearrange("p a b -> p (a b)"))

                # ---- issue schedule ----
                with tc.tile_pool(name="pxs", bufs=2, space="PSUM") as pxs, \
                     tc.tile_pool(name="psp", bufs=1, space="PSUM") as psp, \
                     tc.tile_pool(name="pav", bufs=1, space="PSUM") as pav:
                    pools.update(pxs=pxs, psp=psp, pav=pav)
                    # sel-0 window: tiles (0,0..15); attn(0,half0) from slot 8
                    for i in range(NT):
                        sel = emit_sel_compute(0, i)
                        if i >= 8:
                            for jj in (2 * (i - 8), 2 * (i - 8) + 1):
                                emit_attn(0, 0, jj)
                        emit_sel_mask(0, i, *sel)
                    emit_attn_end(0, 0)
                    # sel-1 window: tiles (1,0..15); attn(0,half1) slots 0-7,
                    # attn(1,half0) slots 8-15
                    for i in range(NT):
                        sel = emit_sel_compute(1, i)
                        if i < 8:
                            for jj in (2 * i, 2 * i + 1):
                                emit_attn(0, 1, jj)
                        else:
                            if i == 8:
                                emit_attn_end(0, 1)
                                emit_C(0)
                            for jj in (2 * (i - 8), 2 * (i - 8) + 1):
                                emit_attn(1, 0, jj)
                        emit_sel_mask(1, i, *sel)
                    emit_attn_end(1, 0)

                # ---- tail: attn(1,half1) on fresh, deeper PSUM pools ----
                with tc.tile_pool(name="ptsp", bufs=2, space="PSUM") as tsp, \
                     tc.tile_pool(name="ptav", bufs=1, space="PSUM") as tav:
                    pools.update(psp=tsp, pav=tav)
                    for j in range(NT):
                        emit_attn(1, 1, j, mul_dve=True)
                    emit_attn_end(1, 1)
                    emit_C(1)

            # ================= out_proj =================
            with tc.tile_pool(name="po", bufs=2) as po, \
                 tc.tile_pool(name="pop", bufs=2, space="PSUM") as pop:
                for i in range(NT):
                    op = pop.tile([128, D], dt.float32, tag="op")
                    for n in range(2):
                        nc.tensor.matmul(op[:, 512 * n:512 * n + 512],
                                         ATcatT[:, 128 * i:128 * i + 128],
                                         wo[:, 512 * n:512 * n + 512])
                    ob = po.tile([128, D], dt.float32, tag="ob")
                    nc.scalar.copy(out=ob[:], in_=op[:])
                    nc.sync.dma_start(out=out_part[128 * i:128 * i + 128, :], in_=ob[:])

    _split_excess_waits(nc, limit=1)
    return nc


def _prep_inputs(hidden_states, Wq, Wk, Wv, Wo, idx_wq, idx_wk):
    hs = np.asarray(hidden_states[0], np.float32)          # [T, D]
    hsT = np.ascontiguousarray(hs.T)                       # [D, T]
    hsT_hi = hsT.astype(BF16)
    hsT_lo = (hsT - hsT_hi.astype(np.float32)).astype(BF16)
    maps = []
    for c in range(NCORES):
        h0, h1 = 2 * c, 2 * c + 1
        Aq_parts, Ak_parts = [], []
        for hh in (h0, h1):
            Wq_h = Wq[64 * hh:64 * hh + 64, :].astype(np.float64)    # [64, D]
            Wk_h = Wk[64 * hh:64 * hh + 64, :].astype(np.float64)
            Aq_parts.append((Wq_h.T @ idx_wq[hh].astype(np.float64)).astype(np.float32))
            Ak_parts.append((Wk_h.T @ idx_wk[hh].astype(np.float64)).astype(np.float32))
        A_cat = np.concatenate(Aq_parts + Ak_parts, axis=1)  # [D, 128]
        A_hi = A_cat.astype(BF16)
        A_lo = (A_cat - A_hi.astype(np.float32)).astype(BF16)

        def qk_chain(hh):
            Wq_h = Wq[64 * hh:64 * hh + 64, :]
            Wk_h = Wk[64 * hh:64 * hh + 64, :]
            return np.concatenate(
                [(Wq_h.T / np.sqrt(DH)).astype(BF16), Wk_h.T.astype(BF16)], axis=1)

        Wv_c = np.concatenate(
            [Wv[64 * h0:64 * h0 + 64, :].T, Wv[64 * h1:64 * h1 + 64, :].T],
            axis=1).astype(BF16)                           # [D, 128]
        WoT_c = np.ascontiguousarray(Wo[:, 64 * h0:64 * h0 + 128].T).astype(BF16)

        maps.append({
            "hsT_hi": hsT_hi,
            "hsT_lo": hsT_lo,
            "A_hi": A_hi,
            "A_lo": A_lo,
            "Wqk_h0": qk_chain(h0),
            "Wqk_h1": qk_chain(h1),
            "Wv_cat": Wv_c,
            "WoT_cat": WoT_c,
        })
    return maps


def kernel(hidden_states, Wq, Wk, Wv, Wo, idx_wq, idx_wk):
    from concourse.bass_utils import run_bass_kernel_spmd

    if "nc" not in _COMPILED:
        _COMPILED["nc"] = _build_module()
    nc = _COMPILED["nc"]

    in_maps = _prep_inputs(np.asarray(hidden_states), np.asarray(Wq),
                           np.asarray(Wk), np.asarray(Wv), np.asarray(Wo),
                           np.asarray(idx_wq), np.asarray(idx_wk))
    res = run_bass_kernel_spmd(nc, in_maps, core_ids=list(range(NCORES)))
    out = np.zeros((T, D), np.float32)
    for c in range(NCORES):
        out += np.asarray(res.results[c]["out_part"], np.float32)
    return out.reshape(B, T, D)


# revision 21
# speedup vs baseline: 1.1392x; 1.0027x over previous
"""DeepSeek sparse attention on 8 Trainium2 NeuronCores.

Head-sharded (2 heads/core). v2 schedule — single Act table set
({Exp, Copy, Sign} all live in exp_and_others -> zero table swaps),
selection/attention software-pipelined:

  - P0: indexer projection I = hs @ A via bf16 hi/lo 3-pass matmul; Q/K/V
    projections bf16, stationary-grouped loops.
  - selection per (h,i): X = iq@ik^T via 96-row hi/lo-stacked bf16 matmul;
    top-32 threshold per query via 16-subset DVE MAX8 + 4 rounds
    max8/match_replace; mask m = Sign(X - t32 + eps) in {-1,+1} on the
    Scalar engine, DMA-transposed to [s,q].
  - attention in [s,q]: e = exp(S^T) (Scalar); w' = e*m (GpSimd TT);
    AV accumulates BOTH e and w' into one PSUM group so av equals
    2*(masked AV) — the ones-row normalization absorbs the factor 2.
    The w'-AV matmuls are emitted one step late so the PE queue never
    head-of-line blocks on the GpSimd multiply.
  - schedule: selection is the DVE-paced spine; attention steps fill
    PE/Act/GpSimd under it: attn(0,half0) under sel-0 slots 8-15,
    attn(0,half1)+attn(1,half0) under sel-1, attn(1,half1) as the tail.
  - out_proj partial per core; host sums the 8 partials.
"""
import sys

sys.path.insert(0, '/opt/trn_rl_repo')
sys.path.insert(0, '/opt/pypackages')

import numpy as np
import ml_dtypes

BF16 = ml_dtypes.bfloat16

B, T, D = 1, 2048, 1024
H, DH, DI, KSEL = 16, 64, 32, 32
NCORES = 8
HPC = H // NCORES
NT = T // 128               # 16 query/key tiles
NK = D // 128               # 8 contraction chunks


MARGIN = 1e-5               # inclusion margin on the threshold

_COMPILED = {}


def _install_ldwopt_patch():
    """bass's walrus invocation pins --enable-ldw-opt=false; this kernel's
    matmul streams reuse the same stationary for consecutive matmuls
    (S^T pairs, AV pairs, P0 groups), so the LDW optimizer saves a large
    fraction of the ~860 LDWEIGHTS on the PE queue. Flip it on."""
    import concourse.bass_utils as bu

    if getattr(bu, "_dsa_ldwopt_patched", False):
        return
    orig = bu.get_walrus_args

    def patched(*args, **kwargs):
        out = orig(*args, **kwargs)
        return [a.replace("--enable-ldw-opt=false", "--enable-ldw-opt=true")
                if isinstance(a, str) else a for a in out]

    bu.get_walrus_args = patched
    bu._dsa_ldwopt_patched = True


def _install_drain_patch():
    import concourse.mybir as mybir
    from concourse.tile import TileContext
    from concourse.vector_clock import ScopedClock

    if getattr(TileContext, "_dsa_patched", False):
        return

    def _patched(self, tick_clock, wait_clock):
        nc = self.nc
        drain_inst = nc.sync.drain()
        wait_clock.add_sem_waits(
            drain_inst.ins, ScopedClock({None: tick_clock.global_clock})
        )
        si = drain_inst.ins.sync_info
        waits = list(si.on_wait or []) if si is not None else []
        if len(waits) > 1:
            drain_inst.ins.sync_info = mybir.SyncInfo(
                on_wait=waits[:1], on_update=list(si.on_update or [])
            )
            for i in range(1, len(waits)):
                extra = nc.sync.drain()
                extra.ins.sync_info = mybir.SyncInfo(
                    on_wait=waits[i:i + 1], on_update=[]
                )
        nc.all_engine_barrier()
        assert self.sems is not None
        popped = nc._tile_sem_poison_stack.pop()
        assert popped is self._sem_poison
        nc.clear_and_free_semaphores(list(self.sems.allocated().values()))
        nc.all_engine_barrier()

    TileContext._drain_and_barrier = _patched
    TileContext._dsa_patched = True


def _split_excess_waits(nc, limit=1):
    """walrus in this container rejects instructions with more sync waits
    than the ISA struct encodes; hoist excess waits onto standalone
    EventSemaphore instructions on the same engine, inserted just before."""
    import concourse.mybir as mybir

    n_new = 0
    for bb in nc.main_func.blocks:
        insts = bb.instructions
        i = 0
        while i < len(insts):
            ins = insts[i]
            si = ins.sync_info
            waits = list(si.on_wait or []) if si is not None else []
            if len(waits) > limit:
                ins.sync_info = mybir.SyncInfo(
                    on_wait=waits[:limit], on_update=list(si.on_update or []))
                pos = i
                for j in range(limit, len(waits), limit):
                    n_new += 1
                    w = mybir.InstEventSemaphore(
                        name=f"WSPLIT-{n_new}", ins=[], outs=[])
                    w.engine = ins.engine
                    w.sync_info = mybir.SyncInfo(
                        on_wait=waits[j:j + limit], on_update=[])
                    nc.register_instruction(w, overwrite=True)
                    insts.insert(pos, w)
                    pos += 1
                    i += 1
            i += 1
    return n_new


def _build_module():
    import concourse.bass as bass
    import concourse.mybir as mybir
    from concourse.tile import TileContext

    _install_drain_patch()
    _install_ldwopt_patch()
    dt = mybir.dt
    nc = bass.Bass()

    hsT_hi = nc.declare_dram_parameter("hsT_hi", [D, T], dt.bfloat16, isOutput=False)
    hsT_lo = nc.declare_dram_parameter("hsT_lo", [D, T], dt.bfloat16, isOutput=False)
    A_hi = nc.declare_dram_parameter("A_hi", [D, 128], dt.bfloat16, isOutput=False)
    A_lo = nc.declare_dram_parameter("A_lo", [D, 128], dt.bfloat16, isOutput=False)
    Wqk_h0 = nc.declare_dram_parameter("Wqk_h0", [D, 128], dt.bfloat16, isOutput=False)
    Wqk_h1 = nc.declare_dram_parameter("Wqk_h1", [D, 128], dt.bfloat16, isOutput=False)
    Wv_cat = nc.declare_dram_parameter("Wv_cat", [D, 128], dt.bfloat16, isOutput=False)
    WoT_cat = nc.declare_dram_parameter("WoT_cat", [128, D], dt.bfloat16, isOutput=False)
    out_part = nc.declare_dram_parameter("out_part", [T, D], dt.bfloat16, isOutput=True)

    Sign = mybir.ActivationFunctionType.Sign
    Exp = mybir.ActivationFunctionType.Exp
    Copy = mybir.ActivationFunctionType.Copy
    MUL = mybir.AluOpType.mult
    SUB = mybir.AluOpType.subtract
    ADD = mybir.AluOpType.add

    with TileContext(nc) as tc:
        with tc.tile_pool(name="state", bufs=1) as st:
            IqSs = [st.tile([96, T], dt.bfloat16, tag=f"IqS{h}", name=f"IqS{h}")
                    for h in range(2)]
            IkSs = [st.tile([96, T], dt.bfloat16, tag=f"IkS{h}", name=f"IkS{h}")
                    for h in range(2)]
            QT = st.tile([128, T], dt.bfloat16, tag="QT")
            KT = st.tile([128, T], dt.bfloat16, tag="KT")
            VP = st.tile([128, NT, 2, 65], dt.bfloat16, tag="VP")
            ATcatT = st.tile([128, T], dt.bfloat16, tag="ATcatT")
            wo = st.tile([128, D], dt.bfloat16, tag="wo")
            nc.sync.dma_start(out=wo[:], in_=WoT_cat[:])

            # ================= P0: projections =================
            with tc.tile_pool(name="hsbp", bufs=1) as hp, \
                 tc.tile_pool(name="p0w", bufs=1) as p0w, \
                 tc.tile_pool(name="p0p", bufs=1, space="PSUM") as p0p:
                hsb = hp.tile([128, NK, T], dt.bfloat16, tag="hsb")
                hslo = hp.tile([128, NK, T], dt.bfloat16, tag="hslo")
                a_h = p0w.tile([128, NK, 128], dt.bfloat16, tag="a_h")
                a_l = p0w.tile([128, NK, 128], dt.bfloat16, tag="a_l")
                qk0_w = p0w.tile([128, NK, 128], dt.bfloat16, tag="qk0_w")
                qk1_w = p0w.tile([128, NK, 128], dt.bfloat16, tag="qk1_w")
                v_w = p0w.tile([128, NK, 128], dt.bfloat16, tag="v_w")

                nc.sync.dma_start(out=a_h[:], in_=A_hi[:].rearrange("(c p) m -> p c m", p=128))
                nc.sync.dma_start(out=a_l[:], in_=A_lo[:].rearrange("(c p) m -> p c m", p=128))
                nc.sync.dma_start(out=qk0_w[:], in_=Wqk_h0[:].rearrange("(c p) m -> p c m", p=128))
                nc.sync.dma_start(out=qk1_w[:], in_=Wqk_h1[:].rearrange("(c p) m -> p c m", p=128))
                nc.sync.dma_start(out=v_w[:], in_=Wv_cat[:].rearrange("(c p) m -> p c m", p=128))
                for k in range(NK):
                    nc.sync.dma_start(out=hsb[:, k, :], in_=hsT_hi[128 * k:128 * k + 128, :])
                    nc.sync.dma_start(out=hslo[:, k, :], in_=hsT_lo[128 * k:128 * k + 128, :])

                ip = p0p.tile([128, T], dt.float32, tag="ip")
                qp = p0p.tile([128, T], dt.float32, tag="qp")
                # pass A: I (hi/lo 3-pass) + QK_h0, LDW-grouped per stationary
                for k in range(NK):
                    for n in range(4):
                        sl = slice(512 * n, 512 * n + 512)
                        nc.tensor.matmul(ip[:, sl], a_h[:, k, :], hsb[:, k, sl],
                                         start=(k == 0), stop=False)
                    for n in range(4):
                        sl = slice(512 * n, 512 * n + 512)
                        nc.tensor.matmul(ip[:, sl], a_h[:, k, :], hslo[:, k, sl],
                                         start=False, stop=False)
                    for n in range(4):
                        sl = slice(512 * n, 512 * n + 512)
                        nc.tensor.matmul(ip[:, sl], a_l[:, k, :], hsb[:, k, sl],
                                         start=False, stop=(k == NK - 1))
                    for n in range(4):
                        sl = slice(512 * n, 512 * n + 512)
                        nc.tensor.matmul(qp[:, sl], qk0_w[:, k, :], hsb[:, k, sl],
                                         start=(k == 0), stop=(k == NK - 1))
                # evac indexer projections: contraction-stacked hi/lo
                for h in range(2):
                    iqr = ip[32 * h:32 * h + 32, :]
                    ikr = ip[64 + 32 * h:64 + 32 * h + 32, :]
                    IqS, IkS = IqSs[h], IkSs[h]
                    nc.scalar.copy(out=IqS[0:32, :], in_=iqr)
                    nc.vector.tensor_copy(IqS[32:64, :], IqS[0:32, :])
                    nc.vector.tensor_tensor(out=IqS[64:96, :], in0=iqr,
                                            in1=IqS[0:32, :], op=SUB)
                    nc.scalar.copy(out=IkS[0:32, :], in_=ikr)
                    nc.vector.tensor_tensor(out=IkS[32:64, :], in0=ikr,
                                            in1=IkS[0:32, :], op=SUB)
                    nc.vector.tensor_copy(IkS[64:96, :], IkS[0:32, :])
                nc.scalar.copy(out=QT[0:64, :], in_=qp[0:64, :])
                nc.scalar.copy(out=KT[0:64, :], in_=qp[64:128, :])

                # pass B, reordered to keep the PE hot: qk1 reuses the qp slot
                # (freed by the fast Act evac above) while the DVE is still
                # building IqS/IkS from ip; V then reuses the ip slot.
                qp1 = p0p.tile([128, T], dt.float32, tag="qp")
                for k in range(NK):
                    for n in range(4):
                        sl = slice(512 * n, 512 * n + 512)
                        nc.tensor.matmul(qp1[:, sl], qk1_w[:, k, :], hsb[:, k, sl],
                                         start=(k == 0), stop=(k == NK - 1))
                nc.scalar.copy(out=QT[64:128, :], in_=qp1[0:64, :])
                nc.scalar.copy(out=KT[64:128, :], in_=qp1[64:128, :])
                vp_ps = p0p.tile([128, T], dt.float32, tag="ip")
                for k in range(NK):
                    for n in range(4):
                        sl = slice(512 * n, 512 * n + 512)
                        nc.tensor.matmul(vp_ps[:, sl], v_w[:, k, :], hsb[:, k, sl],
                                         start=(k == 0), stop=(k == NK - 1))
                # V: rows 0:64 = V_h0^T [dh, s], 64:128 = V_h1^T; DMA-transpose
                vt_b = p0w.tile([128, T], dt.bfloat16, tag="vt_b")
                nc.scalar.copy(out=vt_b[:], in_=vp_ps[:])
                vq = p0w.tile([128, NT, 128], dt.bfloat16, tag="vq")
                nc.sync.dma_start_transpose(out=vq[:], in_=vt_b[:])
                for j in range(NT):
                    for h in range(2):
                        nc.scalar.copy(out=VP[:, j, h, 0:64],
                                       in_=vq[:, j, 64 * h:64 * h + 64])
                nc.vector.memset(VP[:, :, :, 64:65], 1.0)


            # ================= selection + attention pipeline =================
            with tc.tile_pool(name="mtp", bufs=2) as mtp, \
                 tc.tile_pool(name="pa", bufs=2) as pa, \
                 tc.tile_pool(name="pms", bufs=2) as pms, \
                 tc.tile_pool(name="pe_", bufs=2) as pe_, \
                 tc.tile_pool(name="pc", bufs=1) as pc:
                pools = {}

                mts = [mtp.tile([128, NT, NT, 128], dt.bfloat16, tag="mt",
                                name=f"mt{h}") for h in range(2)]
                ats = [pc.tile([128, T], dt.bfloat16, tag=f"at{h}", name=f"at{h}")
                       for h in range(2)]

                def emit_sel_compute(h, i):
                    """X matmuls + candidate top-8s + rounds -> negt; returns
                    (xps, negt) for the deferred mask pass"""
                    cand = pa.tile([128, 128], dt.float32, tag="cand")
                    xps = []
                    for half in range(2):
                        xp = pools["pxs"].tile([128, 1024], dt.float32, tag="xps",
                                      name=f"xp{h}_{i}_{half}")
                        xps.append(xp)
                        for n in range(2):
                            sl = slice(512 * n, 512 * n + 512)
                            c0 = 1024 * half + 512 * n
                            qsl = slice(128 * i, 128 * i + 128)
                            nc.tensor.matmul(xp[:, sl], IqSs[h][:, qsl],
                                             IkSs[h][:, c0:c0 + 512])
                        v8 = xp[:].rearrange("p (s l) -> p l s", l=8)
                        for j in range(8):
                            nc.vector.max(out=cand[:, 64 * half + 8 * j:64 * half + 8 * j + 8],
                                          in_=v8[:, j, :])
                    mx = pa.tile([128, 8], dt.float32, tag="mx")
                    for r in range(4):
                        nc.vector.max(out=mx[:], in_=cand[:])
                        if r < 3:
                            nc.vector.match_replace(out=cand[:], in_to_replace=mx[:],
                                                    in_values=cand[:], imm_value=-1e30)
                    # negt = -t32 + margin; m = Sign(X + negt) in {-1,+1}
                    # (Sign lives in exp_and_others with Exp/Copy: no table swap)
                    negt = pa.tile([128, 1], dt.float32, tag="negt")
                    nc.vector.tensor_scalar(negt[:], mx[:, 7:8], -1.0,
                                            scalar2=MARGIN, op0=MUL, op1=ADD)
                    return xps, negt

                def emit_sel_mask(h, i, xps, negt):
                    """Sign masks + transpose. Emitted AFTER the slot's attn
                    exps so the Act queue never head-of-line blocks on negt."""
                    mt = mts[h]
                    ms = pms.tile([128, T], dt.bfloat16, tag="ms", name=f"ms{h}_{i}")
                    for half in range(2):
                        nc.scalar.activation(
                            out=ms[:, 1024 * half:1024 * half + 1024],
                            in_=xps[half][:], func=Sign, bias=negt[:])
                    nc.sync.dma_start_transpose(out=mt[:, :, i, :], in_=ms[:])

                # per-stream state: current av tile + the lagged w'-AV emission
                avst = {"av": None, "pending": None}

                def flush_wav(last=False):
                    """emit the lagged w'-AV matmuls for the previous step"""
                    p = avst["pending"]
                    if p is None:
                        return
                    avst["pending"] = None
                    av, wt, h, j = p
                    for n in range(2):
                        sl = slice(512 * n, 512 * n + 512)
                        nc.tensor.matmul(av[:, sl], VP[:, j, h, :], wt[:, sl],
                                         start=False, stop=last)

                def emit_attn(h, half, j, mul_dve=False):
                    """attention step: sp = K_j^T Q_half; e = exp(sp);
                    w' = e * sign-mask (GpSimd TT, or DVE TT in the tail where
                    DVE is idle); av += VP@(e) and += VP@(w') so av equals
                    2*(masked AV) — the ones-row normalization absorbs it.
                    The w'-AV matmuls are emitted one step late so the PE queue
                    never head-of-line blocks on the mask multiply. A dummy
                    LDWEIGHTS dep'd on e fires mid-slot to keep the PE HAM
                    activity monitor fed (cold PE runs at half clock)."""
                    mt = mts[h]
                    sp = pools["psp"].tile([128, 1024], dt.float32, tag="sp",
                                           name=f"sp{h}_{half}_{j}")
                    for n in range(2):
                        sl = slice(512 * n, 512 * n + 512)
                        c0 = 1024 * half + 512 * n
                        nc.tensor.matmul(sp[:, sl],
                                         KT[64 * h:64 * h + 64, 128 * j:128 * j + 128],
                                         QT[64 * h:64 * h + 64, c0:c0 + 512])
                    e = pe_.tile([128, 1024], dt.bfloat16, tag="e", name="e")
                    nc.scalar.activation(out=e[:], in_=sp[:], func=Exp)
                    msl = mt[:, j, 8 * half:8 * half + 8, :].rearrange("p a b -> p (a b)")
                    wt = pe_.tile([128, 1024], dt.bfloat16, tag="w", name="w")
                    if mul_dve:
                        nc.vector.tensor_tensor(out=wt[:], in0=e[:], in1=msl, op=MUL)
                    else:
                        nc.gpsimd.tensor_tensor(out=wt[:], in0=e[:], in1=msl, op=MUL)
                    if j == 0:
                        avst["av"] = pools["pav"].tile([65, 1024], dt.float32, tag="av",
                                                       name=f"av{h}_{half}")
                    av = avst["av"]
                    for n in range(2):
                        sl = slice(512 * n, 512 * n + 512)
                        nc.tensor.matmul(av[:, sl], VP[:, j, h, :], e[:, sl],
                                         start=(j == 0), stop=False)
                    flush_wav()
                    avst["pending"] = (av, wt, h, j)

                def emit_attn_end(h, half):
                    flush_wav(last=True)
                    nc.scalar.copy(out=ats[h][0:65, 1024 * half:1024 * half + 1024],
                                   in_=avst["av"][:])

                def emit_C(h):
                    """normalize + build transposed attn rows of ATcatT"""
                    at = ats[h]
                    atq = pc.tile([128, NT, 128], dt.bfloat16, tag="atq")
                    nc.sync.dma_start_transpose(out=atq[:], in_=at[:])
                    scrall = pc.tile([128, NT, 128], dt.bfloat16, tag="scrall")
                    rds = pa.tile([128, NT], dt.float32, tag="rds")
                    nc.vector.reciprocal(
                        rds[:], atq[:, :, 64:65].rearrange("p a b -> p (a b)"))
                    for i in range(NT):
                        nc.scalar.activation(out=scrall[:, i, 0:64],
                                             in_=atq[:, i, 0:64], func=Copy,
                                             scale=rds[:, i:i + 1])
                    tmpT = pc.tile([128, NT, 128], dt.bfloat16, tag="tmpT")
                    nc.sync.dma_start_transpose(out=tmpT[:], in_=scrall[:])
                    nc.vector.tensor_copy(ATcatT[64 * h:64 * h + 64, :],
                                          tmpT[0:64, :, :].rearrange("p a b -> p (a b)"))

                # ---- issue schedule ----
                with tc.tile_pool(name="pxs", bufs=2, space="PSUM") as pxs, \
                     tc.tile_pool(name="psp", bufs=1, space="PSUM") as psp, \
                     tc.tile_pool(name="pav", bufs=1, space="PSUM") as pav:
                    pools.update(pxs=pxs, psp=psp, pav=pav)
                    # sel-0 window: tiles (0,0..15); attn(0,half0) from slot 8
                    for i in range(NT):
                        sel = emit_sel_compute(0, i)
                        if i >= 8:
                            for jj in (2 * (i - 8), 2 * (i - 8) + 1):
                                emit_attn(0, 0, jj)
                        emit_sel_mask(0, i, *sel)
                    emit_attn_end(0, 0)
                    # sel-1 window: tiles (1,0..15); attn(0,half1) slots 0-7,
                    # attn(1,half0) slots 8-15
                    for i in range(NT):
                        sel = emit_sel_compute(1, i)
                        if i < 8:
                            for jj in (2 * i, 2 * i + 1):
                                emit_attn(0, 1, jj)
                        else:
                            if i == 8:
                                emit_attn_end(0, 1)
                                emit_C(0)
                            for jj in (2 * (i - 8), 2 * (i - 8) + 1):
                                emit_attn(1, 0, jj)
                        emit_sel_mask(1, i, *sel)
                    emit_attn_end(1, 0)

                # ---- tail: attn(1,half1) on fresh, deeper PSUM pools ----
                with tc.tile_pool(name="ptsp", bufs=2, space="PSUM") as tsp, \
                     tc.tile_pool(name="ptav", bufs=1, space="PSUM") as tav:
                    pools.update(psp=tsp, pav=tav)
                    for j in range(NT):
                        emit_attn(1, 1, j, mul_dve=True)
                    emit_attn_end(1, 1)
                    emit_C(1)

            # ================= out_proj =================
            with tc.tile_pool(name="po", bufs=2) as po, \
                 tc.tile_pool(name="pop", bufs=2, space="PSUM") as pop:
                for i in range(NT):
                    op = pop.tile([128, D], dt.float32, tag="op")
                    for n in range(2):
                        nc.tensor.matmul(op[:, 512 * n:512 * n + 512],
                                         ATcatT[:, 128 * i:128 * i + 128],
                                         wo[:, 512 * n:512 * n + 512])
                    ob = po.tile([128, D], dt.bfloat16, tag="ob")
                    nc.scalar.copy(out=ob[:], in_=op[:])
                    nc.sync.dma_start(out=out_part[128 * i:128 * i + 128, :], in_=ob[:])

    _split_excess_waits(nc, limit=1)
    return nc


def _prep_inputs(hidden_states, Wq, Wk, Wv, Wo, idx_wq, idx_wk):
    hs = np.asarray(hidden_states[0], np.float32)          # [T, D]
    hsT = np.ascontiguousarray(hs.T)                       # [D, T]
    hsT_hi = hsT.astype(BF16)
    hsT_lo = (hsT - hsT_hi.astype(np.float32)).astype(BF16)
    maps = []
    for c in range(NCORES):
        h0, h1 = 2 * c, 2 * c + 1
        Aq_parts, Ak_parts = [], []
        for hh in (h0, h1):
            Wq_h = Wq[64 * hh:64 * hh + 64, :].astype(np.float64)    # [64, D]
            Wk_h = Wk[64 * hh:64 * hh + 64, :].astype(np.float64)
            Aq_parts.append((Wq_h.T @ idx_wq[hh].astype(np.float64)).astype(np.float32))
            Ak_parts.append((Wk_h.T @ idx_wk[hh].astype(np.float64)).astype(np.float32))
        A_cat = np.concatenate(Aq_parts + Ak_parts, axis=1)  # [D, 128]
        A_hi = A_cat.astype(BF16)
        A_lo = (A_cat - A_hi.astype(np.float32)).astype(BF16)

        def qk_chain(hh):
            Wq_h = Wq[64 * hh:64 * hh + 64, :]
            Wk_h = Wk[64 * hh:64 * hh + 64, :]
            return np.concatenate(
                [(Wq_h.T / np.sqrt(DH)).astype(BF16), Wk_h.T.astype(BF16)], axis=1)

        Wv_c = np.concatenate(
            [Wv[64 * h0:64 * h0 + 64, :].T, Wv[64 * h1:64 * h1 + 64, :].T],
            axis=1).astype(BF16)                           # [D, 128]
        WoT_c = np.ascontiguousarray(Wo[:, 64 * h0:64 * h0 + 128].T).astype(BF16)

        maps.append({
            "hsT_hi": hsT_hi,
            "hsT_lo": hsT_lo,
            "A_hi": A_hi,
            "A_lo": A_lo,
            "Wqk_h0": qk_chain(h0),
            "Wqk_h1": qk_chain(h1),
            "Wv_cat": Wv_c,
            "WoT_cat": WoT_c,
        })
    return maps


def kernel(hidden_states, Wq, Wk, Wv, Wo, idx_wq, idx_wk):
    from concourse.bass_utils import run_bass_kernel_spmd

    if "nc" not in _COMPILED:
        _COMPILED["nc"] = _build_module()
    nc = _COMPILED["nc"]

    in_maps = _prep_inputs(np.asarray(hidden_states), np.asarray(Wq),
                           np.asarray(Wk), np.asarray(Wv), np.asarray(Wo),
                           np.asarray(idx_wq), np.asarray(idx_wk))
    res = run_bass_kernel_spmd(nc, in_maps, core_ids=list(range(NCORES)))
    out = np.zeros((T, D), np.float32)
    for c in range(NCORES):
        out += np.asarray(res.results[c]["out_part"], np.float32)
    return out.reshape(B, T, D)


# revision 22
# speedup vs baseline: 1.1413x; 1.0019x over previous
"""DeepSeek sparse attention on 8 Trainium2 NeuronCores.

Head-sharded (2 heads/core). v2 schedule — single Act table set
({Exp, Copy, Sign} all live in exp_and_others -> zero table swaps),
selection/attention software-pipelined:

  - P0: indexer projection I = hs @ A via bf16 hi/lo 3-pass matmul; Q/K/V
    projections bf16, stationary-grouped loops.
  - selection per (h,i): X = iq@ik^T via 96-row hi/lo-stacked bf16 matmul;
    top-32 threshold per query via 16-subset DVE MAX8 + 4 rounds
    max8/match_replace; mask m = Sign(X - t32 + eps) in {-1,+1} on the
    Scalar engine, DMA-transposed to [s,q].
  - attention in [s,q]: e = exp(S^T) (Scalar); w' = e*m (GpSimd TT);
    AV accumulates BOTH e and w' into one PSUM group so av equals
    2*(masked AV) — the ones-row normalization absorbs the factor 2.
    The w'-AV matmuls are emitted one step late so the PE queue never
    head-of-line blocks on the GpSimd multiply.
  - schedule: selection is the DVE-paced spine; attention steps fill
    PE/Act/GpSimd under it: attn(0,half0) under sel-0 slots 8-15,
    attn(0,half1)+attn(1,half0) under sel-1, attn(1,half1) as the tail.
  - out_proj partial per core; host sums the 8 partials.
"""
import sys

sys.path.insert(0, '/opt/trn_rl_repo')
sys.path.insert(0, '/opt/pypackages')

import numpy as np
import ml_dtypes

BF16 = ml_dtypes.bfloat16

B, T, D = 1, 2048, 1024
H, DH, DI, KSEL = 16, 64, 32, 32
NCORES = 8
HPC = H // NCORES
NT = T // 128               # 16 query/key tiles
NK = D // 128               # 8 contraction chunks


MARGIN = 1e-5               # inclusion margin on the threshold

_COMPILED = {}


def _install_ldwopt_patch():
    """bass's walrus invocation pins --enable-ldw-opt=false; this kernel's
    matmul streams reuse the same stationary for consecutive matmuls
    (S^T pairs, AV pairs, P0 groups), so the LDW optimizer saves a large
    fraction of the ~860 LDWEIGHTS on the PE queue. Flip it on."""
    import concourse.bass_utils as bu

    if getattr(bu, "_dsa_ldwopt_patched", False):
        return
    orig = bu.get_walrus_args

    def patched(*args, **kwargs):
        out = orig(*args, **kwargs)
        return [a.replace("--enable-ldw-opt=false", "--enable-ldw-opt=true")
                if isinstance(a, str) else a for a in out]

    bu.get_walrus_args = patched
    bu._dsa_ldwopt_patched = True


def _install_drain_patch():
    import concourse.mybir as mybir
    from concourse.tile import TileContext
    from concourse.vector_clock import ScopedClock

    if getattr(TileContext, "_dsa_patched", False):
        return

    def _patched(self, tick_clock, wait_clock):
        nc = self.nc
        drain_inst = nc.sync.drain()
        wait_clock.add_sem_waits(
            drain_inst.ins, ScopedClock({None: tick_clock.global_clock})
        )
        si = drain_inst.ins.sync_info
        waits = list(si.on_wait or []) if si is not None else []
        if len(waits) > 1:
            drain_inst.ins.sync_info = mybir.SyncInfo(
                on_wait=waits[:1], on_update=list(si.on_update or [])
            )
            for i in range(1, len(waits)):
                extra = nc.sync.drain()
                extra.ins.sync_info = mybir.SyncInfo(
                    on_wait=waits[i:i + 1], on_update=[]
                )
        nc.all_engine_barrier()
        assert self.sems is not None
        popped = nc._tile_sem_poison_stack.pop()
        assert popped is self._sem_poison
        nc.clear_and_free_semaphores(list(self.sems.allocated().values()))
        nc.all_engine_barrier()

    TileContext._drain_and_barrier = _patched
    TileContext._dsa_patched = True


def _split_excess_waits(nc, limit=1):
    """walrus in this container rejects instructions with more sync waits
    than the ISA struct encodes; hoist excess waits onto standalone
    EventSemaphore instructions on the same engine, inserted just before."""
    import concourse.mybir as mybir

    n_new = 0
    for bb in nc.main_func.blocks:
        insts = bb.instructions
        i = 0
        while i < len(insts):
            ins = insts[i]
            si = ins.sync_info
            waits = list(si.on_wait or []) if si is not None else []
            if len(waits) > limit:
                ins.sync_info = mybir.SyncInfo(
                    on_wait=waits[:limit], on_update=list(si.on_update or []))
                pos = i
                for j in range(limit, len(waits), limit):
                    n_new += 1
                    w = mybir.InstEventSemaphore(
                        name=f"WSPLIT-{n_new}", ins=[], outs=[])
                    w.engine = ins.engine
                    w.sync_info = mybir.SyncInfo(
                        on_wait=waits[j:j + limit], on_update=[])
                    nc.register_instruction(w, overwrite=True)
                    insts.insert(pos, w)
                    pos += 1
                    i += 1
            i += 1
    return n_new


def _build_module():
    import concourse.bass as bass
    import concourse.mybir as mybir
    from concourse.tile import TileContext

    _install_drain_patch()
    _install_ldwopt_patch()
    dt = mybir.dt
    nc = bass.Bass()

    hsT_hi = nc.declare_dram_parameter("hsT_hi", [D, T], dt.bfloat16, isOutput=False)
    hsT_lo = nc.declare_dram_parameter("hsT_lo", [D, T], dt.bfloat16, isOutput=False)
    A_hi = nc.declare_dram_parameter("A_hi", [D, 128], dt.bfloat16, isOutput=False)
    A_lo = nc.declare_dram_parameter("A_lo", [D, 128], dt.bfloat16, isOutput=False)
    Wqk_h0 = nc.declare_dram_parameter("Wqk_h0", [D, 128], dt.bfloat16, isOutput=False)
    Wqk_h1 = nc.declare_dram_parameter("Wqk_h1", [D, 128], dt.bfloat16, isOutput=False)
    Wv_cat = nc.declare_dram_parameter("Wv_cat", [D, 128], dt.bfloat16, isOutput=False)
    WoT_cat = nc.declare_dram_parameter("WoT_cat", [128, D], dt.bfloat16, isOutput=False)
    out_part = nc.declare_dram_parameter("out_part", [T, D], dt.bfloat16, isOutput=True)

    Sign = mybir.ActivationFunctionType.Sign
    Exp = mybir.ActivationFunctionType.Exp
    Copy = mybir.ActivationFunctionType.Copy
    MUL = mybir.AluOpType.mult
    SUB = mybir.AluOpType.subtract
    ADD = mybir.AluOpType.add

    with TileContext(nc) as tc:
        with tc.tile_pool(name="state", bufs=1) as st:
            IqSs = [st.tile([96, T], dt.bfloat16, tag=f"IqS{h}", name=f"IqS{h}")
                    for h in range(2)]
            IkSs = [st.tile([96, T], dt.bfloat16, tag=f"IkS{h}", name=f"IkS{h}")
                    for h in range(2)]
            QT = st.tile([128, T], dt.bfloat16, tag="QT")
            KT = st.tile([128, T], dt.bfloat16, tag="KT")
            VP = st.tile([128, NT, 2, 65], dt.bfloat16, tag="VP")
            ATcatT = st.tile([128, T], dt.bfloat16, tag="ATcatT")
            wo = st.tile([128, D], dt.bfloat16, tag="wo")
            nc.sync.dma_start(out=wo[:], in_=WoT_cat[:])

            # ================= P0: projections =================
            with tc.tile_pool(name="hsbp", bufs=1) as hp, \
                 tc.tile_pool(name="p0w", bufs=1) as p0w, \
                 tc.tile_pool(name="p0p", bufs=1, space="PSUM") as p0p:
                hsb = hp.tile([128, NK, T], dt.bfloat16, tag="hsb")
                hslo = hp.tile([128, NK, T], dt.bfloat16, tag="hslo")
                a_h = p0w.tile([128, NK, 128], dt.bfloat16, tag="a_h")
                a_l = p0w.tile([128, NK, 128], dt.bfloat16, tag="a_l")
                qk0_w = p0w.tile([128, NK, 128], dt.bfloat16, tag="qk0_w")
                qk1_w = p0w.tile([128, NK, 128], dt.bfloat16, tag="qk1_w")
                v_w = p0w.tile([128, NK, 128], dt.bfloat16, tag="v_w")

                nc.sync.dma_start(out=a_h[:], in_=A_hi[:].rearrange("(c p) m -> p c m", p=128))
                nc.sync.dma_start(out=a_l[:], in_=A_lo[:].rearrange("(c p) m -> p c m", p=128))
                nc.sync.dma_start(out=qk0_w[:], in_=Wqk_h0[:].rearrange("(c p) m -> p c m", p=128))
                nc.sync.dma_start(out=qk1_w[:], in_=Wqk_h1[:].rearrange("(c p) m -> p c m", p=128))
                nc.sync.dma_start(out=v_w[:], in_=Wv_cat[:].rearrange("(c p) m -> p c m", p=128))
                for k in range(NK):
                    nc.sync.dma_start(out=hsb[:, k, :], in_=hsT_hi[128 * k:128 * k + 128, :])
                    nc.sync.dma_start(out=hslo[:, k, :], in_=hsT_lo[128 * k:128 * k + 128, :])

                ip = p0p.tile([128, T], dt.float32, tag="ip")
                qp = p0p.tile([128, T], dt.float32, tag="qp")
                # pass A: I (hi/lo 3-pass) + QK_h0, LDW-grouped per stationary
                for k in range(NK):
                    for n in range(4):
                        sl = slice(512 * n, 512 * n + 512)
                        nc.tensor.matmul(ip[:, sl], a_h[:, k, :], hsb[:, k, sl],
                                         start=(k == 0), stop=False)
                    for n in range(4):
                        sl = slice(512 * n, 512 * n + 512)
                        nc.tensor.matmul(ip[:, sl], a_h[:, k, :], hslo[:, k, sl],
                                         start=False, stop=False)
                    for n in range(4):
                        sl = slice(512 * n, 512 * n + 512)
                        nc.tensor.matmul(ip[:, sl], a_l[:, k, :], hsb[:, k, sl],
                                         start=False, stop=(k == NK - 1))
                    for n in range(4):
                        sl = slice(512 * n, 512 * n + 512)
                        nc.tensor.matmul(qp[:, sl], qk0_w[:, k, :], hsb[:, k, sl],
                                         start=(k == 0), stop=(k == NK - 1))
                # evac indexer projections: contraction-stacked hi/lo
                for h in range(2):
                    iqr = ip[32 * h:32 * h + 32, :]
                    ikr = ip[64 + 32 * h:64 + 32 * h + 32, :]
                    IqS, IkS = IqSs[h], IkSs[h]
                    nc.scalar.copy(out=IqS[0:32, :], in_=iqr)
                    nc.vector.tensor_copy(IqS[32:64, :], IqS[0:32, :])
                    nc.vector.tensor_tensor(out=IqS[64:96, :], in0=iqr,
                                            in1=IqS[0:32, :], op=SUB)
                    nc.scalar.copy(out=IkS[0:32, :], in_=ikr)
                    nc.vector.tensor_tensor(out=IkS[32:64, :], in0=ikr,
                                            in1=IkS[0:32, :], op=SUB)
                    nc.vector.tensor_copy(IkS[64:96, :], IkS[0:32, :])
                nc.scalar.copy(out=QT[0:64, :], in_=qp[0:64, :])
                nc.scalar.copy(out=KT[0:64, :], in_=qp[64:128, :])

                # pass B, reordered to keep the PE hot: qk1 reuses the qp slot
                # (freed by the fast Act evac above) while the DVE is still
                # building IqS/IkS from ip; V then reuses the ip slot.
                qp1 = p0p.tile([128, T], dt.float32, tag="qp")
                for k in range(NK):
                    for n in range(4):
                        sl = slice(512 * n, 512 * n + 512)
                        nc.tensor.matmul(qp1[:, sl], qk1_w[:, k, :], hsb[:, k, sl],
                                         start=(k == 0), stop=(k == NK - 1))
                nc.scalar.copy(out=QT[64:128, :], in_=qp1[0:64, :])
                nc.scalar.copy(out=KT[64:128, :], in_=qp1[64:128, :])
                vp_ps = p0p.tile([128, T], dt.float32, tag="ip")
                for k in range(NK):
                    for n in range(4):
                        sl = slice(512 * n, 512 * n + 512)
                        nc.tensor.matmul(vp_ps[:, sl], v_w[:, k, :], hsb[:, k, sl],
                                         start=(k == 0), stop=(k == NK - 1))
                # V: rows 0:64 = V_h0^T [dh, s], 64:128 = V_h1^T; DMA-transpose
                vt_b = p0w.tile([128, T], dt.bfloat16, tag="vt_b")
                nc.scalar.copy(out=vt_b[:], in_=vp_ps[:])
                vq = p0w.tile([128, NT, 128], dt.bfloat16, tag="vq")
                nc.sync.dma_start_transpose(out=vq[:], in_=vt_b[:])
                for j in range(NT):
                    for h in range(2):
                        nc.scalar.copy(out=VP[:, j, h, 0:64],
                                       in_=vq[:, j, 64 * h:64 * h + 64])
                nc.vector.memset(VP[:, :, :, 64:65], 1.0)


            # ================= selection + attention pipeline =================
            with tc.tile_pool(name="mtp", bufs=2) as mtp, \
                 tc.tile_pool(name="pa", bufs=2) as pa, \
                 tc.tile_pool(name="pms", bufs=2) as pms, \
                 tc.tile_pool(name="pe_", bufs=2) as pe_, \
                 tc.tile_pool(name="pc", bufs=1) as pc:
                pools = {}

                mts = [mtp.tile([128, NT, NT, 128], dt.bfloat16, tag="mt",
                                name=f"mt{h}") for h in range(2)]
                ats = [pc.tile([128, T], dt.bfloat16, tag=f"at{h}", name=f"at{h}")
                       for h in range(2)]

                def emit_sel_compute(h, i):
                    """X matmuls + candidate top-8s + rounds -> negt; returns
                    (xps, negt) for the deferred mask pass"""
                    cand = pa.tile([128, 128], dt.float32, tag="cand")
                    xps = []
                    for half in range(2):
                        xp = pools["pxs"].tile([128, 1024], dt.float32, tag="xps",
                                      name=f"xp{h}_{i}_{half}")
                        xps.append(xp)
                        for n in range(2):
                            sl = slice(512 * n, 512 * n + 512)
                            c0 = 1024 * half + 512 * n
                            qsl = slice(128 * i, 128 * i + 128)
                            nc.tensor.matmul(xp[:, sl], IqSs[h][:, qsl],
                                             IkSs[h][:, c0:c0 + 512])
                        v8 = xp[:].rearrange("p (s l) -> p l s", l=8)
                        for j in range(8):
                            nc.vector.max(out=cand[:, 64 * half + 8 * j:64 * half + 8 * j + 8],
                                          in_=v8[:, j, :])
                    mx = pa.tile([128, 8], dt.float32, tag="mx")
                    for r in range(4):
                        nc.vector.max(out=mx[:], in_=cand[:])
                        if r < 3:
                            nc.vector.match_replace(out=cand[:], in_to_replace=mx[:],
                                                    in_values=cand[:], imm_value=-1e30)
                    # negt = -t32 + margin; m = Sign(X + negt) in {-1,+1}
                    # (Sign lives in exp_and_others with Exp/Copy: no table swap)
                    negt = pa.tile([128, 1], dt.float32, tag="negt")
                    nc.vector.tensor_scalar(negt[:], mx[:, 7:8], -1.0,
                                            scalar2=MARGIN, op0=MUL, op1=ADD)
                    return xps, negt

                def emit_sel_mask(h, i, xps, negt):
                    """Sign masks + transpose. Emitted AFTER the slot's attn
                    exps so the Act queue never head-of-line blocks on negt."""
                    mt = mts[h]
                    ms = pms.tile([128, T], dt.bfloat16, tag="ms", name=f"ms{h}_{i}")
                    for half in range(2):
                        nc.scalar.activation(
                            out=ms[:, 1024 * half:1024 * half + 1024],
                            in_=xps[half][:], func=Sign, bias=negt[:])
                    nc.sync.dma_start_transpose(out=mt[:, :, i, :], in_=ms[:])

                # per-stream state: current av tile + the lagged w'-AV emission
                avst = {"av": None, "pending": None}

                def flush_wav(last=False):
                    """emit the lagged w'-AV matmuls for the previous step"""
                    p = avst["pending"]
                    if p is None:
                        return
                    avst["pending"] = None
                    av, wt, h, j = p
                    for n in range(2):
                        sl = slice(512 * n, 512 * n + 512)
                        nc.tensor.matmul(av[:, sl], VP[:, j, h, :], wt[:, sl],
                                         start=False, stop=last)

                def emit_attn(h, half, j, mul_dve=False):
                    """attention step: sp = K_j^T Q_half; e = exp(sp);
                    w' = e * sign-mask (GpSimd TT, or DVE TT in the tail where
                    DVE is idle); av += VP@(e) and += VP@(w') so av equals
                    2*(masked AV) — the ones-row normalization absorbs it.
                    The w'-AV matmuls are emitted one step late so the PE queue
                    never head-of-line blocks on the mask multiply. A dummy
                    LDWEIGHTS dep'd on e fires mid-slot to keep the PE HAM
                    activity monitor fed (cold PE runs at half clock)."""
                    mt = mts[h]
                    sp = pools["psp"].tile([128, 1024], dt.float32, tag="sp",
                                           name=f"sp{h}_{half}_{j}")
                    for n in range(2):
                        sl = slice(512 * n, 512 * n + 512)
                        c0 = 1024 * half + 512 * n
                        nc.tensor.matmul(sp[:, sl],
                                         KT[64 * h:64 * h + 64, 128 * j:128 * j + 128],
                                         QT[64 * h:64 * h + 64, c0:c0 + 512])
                    e = pe_.tile([128, 1024], dt.bfloat16, tag="e", name="e")
                    nc.scalar.activation(out=e[:], in_=sp[:], func=Exp)
                    msl = mt[:, j, 8 * half:8 * half + 8, :].rearrange("p a b -> p (a b)")
                    wt = pe_.tile([128, 1024], dt.bfloat16, tag="w", name="w")
                    if mul_dve:
                        nc.vector.tensor_tensor(out=wt[:], in0=e[:], in1=msl, op=MUL)
                    else:
                        nc.gpsimd.tensor_tensor(out=wt[:], in0=e[:], in1=msl, op=MUL)
                    if j == 0:
                        avst["av"] = pools["pav"].tile([65, 1024], dt.float32, tag="av",
                                                       name=f"av{h}_{half}")
                    av = avst["av"]
                    for n in range(2):
                        sl = slice(512 * n, 512 * n + 512)
                        nc.tensor.matmul(av[:, sl], VP[:, j, h, :], e[:, sl],
                                         start=(j == 0), stop=False)
                    flush_wav()
                    avst["pending"] = (av, wt, h, j)

                def emit_attn_end(h, half):
                    flush_wav(last=True)
                    nc.scalar.copy(out=ats[h][0:65, 1024 * half:1024 * half + 1024],
                                   in_=avst["av"][:])

                def emit_C(h):
                    """normalize + build transposed attn rows of ATcatT"""
                    at = ats[h]
                    atq = pc.tile([128, NT, 128], dt.bfloat16, tag="atq")
                    nc.sync.dma_start_transpose(out=atq[:], in_=at[:])
                    scrall = pc.tile([128, NT, 128], dt.bfloat16, tag="scrall")
                    rds = pa.tile([128, NT], dt.float32, tag="rds")
                    nc.vector.reciprocal(
                        rds[:], atq[:, :, 64:65].rearrange("p a b -> p (a b)"))
                    for i in range(NT):
                        nc.scalar.activation(out=scrall[:, i, 0:64],
                                             in_=atq[:, i, 0:64], func=Copy,
                                             scale=rds[:, i:i + 1])
                    tmpT = pc.tile([128, NT, 128], dt.bfloat16, tag="tmpT")
                    nc.sync.dma_start_transpose(out=tmpT[:], in_=scrall[:])
                    nc.vector.tensor_copy(ATcatT[64 * h:64 * h + 64, :],
                                          tmpT[0:64, :, :].rearrange("p a b -> p (a b)"))

                # ---- issue schedule ----
                with tc.tile_pool(name="pxs", bufs=2, space="PSUM") as pxs, \
                     tc.tile_pool(name="psp", bufs=1, space="PSUM") as psp, \
                     tc.tile_pool(name="pav", bufs=1, space="PSUM") as pav:
                    pools.update(pxs=pxs, psp=psp, pav=pav)
                    # sel-0 window: tiles (0,0..15); attn(0,half0) from slot 8
                    for i in range(NT):
                        sel = emit_sel_compute(0, i)
                        if i >= 8:
                            for jj in (2 * (i - 8), 2 * (i - 8) + 1):
                                emit_attn(0, 0, jj)
                        emit_sel_mask(0, i, *sel)
                    emit_attn_end(0, 0)
                    # sel-1 window: tiles (1,0..15); attn(0,half1) slots 0-7,
                    # attn(1,half0) slots 8-15
                    for i in range(NT):
                        sel = emit_sel_compute(1, i)
                        if i < 8:
                            for jj in (2 * i, 2 * i + 1):
                                emit_attn(0, 1, jj)
                        else:
                            if i == 8:
                                emit_attn_end(0, 1)
                            for jj in (2 * (i - 8), 2 * (i - 8) + 1):
                                emit_attn(1, 0, jj)
                        emit_sel_mask(1, i, *sel)
                    emit_attn_end(1, 0)

                # ---- tail: attn(1,half1) on fresh, deeper PSUM pools ----
                with tc.tile_pool(name="ptsp", bufs=2, space="PSUM") as tsp, \
                     tc.tile_pool(name="ptav", bufs=1, space="PSUM") as tav:
                    pools.update(psp=tsp, pav=tav)
                    for j in range(NT):
                        emit_attn(1, 1, j, mul_dve=True)
                        if j == 2:
                            emit_C(0)   # Act/DVE have slack here, not mid-window
                    emit_attn_end(1, 1)
                    emit_C(1)

            # ================= out_proj =================
            with tc.tile_pool(name="po", bufs=2) as po, \
                 tc.tile_pool(name="pop", bufs=2, space="PSUM") as pop:
                for i in range(NT):
                    op = pop.tile([128, D], dt.float32, tag="op")
                    for n in range(2):
                        nc.tensor.matmul(op[:, 512 * n:512 * n + 512],
                                         ATcatT[:, 128 * i:128 * i + 128],
                                         wo[:, 512 * n:512 * n + 512])
                    ob = po.tile([128, D], dt.bfloat16, tag="ob")
                    nc.scalar.copy(out=ob[:], in_=op[:])
                    nc.sync.dma_start(out=out_part[128 * i:128 * i + 128, :], in_=ob[:])

    _split_excess_waits(nc, limit=1)
    return nc


def _prep_inputs(hidden_states, Wq, Wk, Wv, Wo, idx_wq, idx_wk):
    hs = np.asarray(hidden_states[0], np.float32)          # [T, D]
    hsT = np.ascontiguousarray(hs.T)                       # [D, T]
    hsT_hi = hsT.astype(BF16)
    hsT_lo = (hsT - hsT_hi.astype(np.float32)).astype(BF16)
    maps = []
    for c in range(NCORES):
        h0, h1 = 2 * c, 2 * c + 1
        Aq_parts, Ak_parts = [], []
        for hh in (h0, h1):
            Wq_h = Wq[64 * hh:64 * hh + 64, :].astype(np.float64)    # [64, D]
            Wk_h = Wk[64 * hh:64 * hh + 64, :].astype(np.float64)
            Aq_parts.append((Wq_h.T @ idx_wq[hh].astype(np.float64)).astype(np.float32))
            Ak_parts.append((Wk_h.T @ idx_wk[hh].astype(np.float64)).astype(np.float32))
        A_cat = np.concatenate(Aq_parts + Ak_parts, axis=1)  # [D, 128]
        A_hi = A_cat.astype(BF16)
        A_lo = (A_cat - A_hi.astype(np.float32)).astype(BF16)

        def qk_chain(hh):
            Wq_h = Wq[64 * hh:64 * hh + 64, :]
            Wk_h = Wk[64 * hh:64 * hh + 64, :]
            return np.concatenate(
                [(Wq_h.T / np.sqrt(DH)).astype(BF16), Wk_h.T.astype(BF16)], axis=1)

        Wv_c = np.concatenate(
            [Wv[64 * h0:64 * h0 + 64, :].T, Wv[64 * h1:64 * h1 + 64, :].T],
            axis=1).astype(BF16)                           # [D, 128]
        WoT_c = np.ascontiguousarray(Wo[:, 64 * h0:64 * h0 + 128].T).astype(BF16)

        maps.append({
            "hsT_hi": hsT_hi,
            "hsT_lo": hsT_lo,
            "A_hi": A_hi,
            "A_lo": A_lo,
            "Wqk_h0": qk_chain(h0),
            "Wqk_h1": qk_chain(h1),
            "Wv_cat": Wv_c,
            "WoT_cat": WoT_c,
        })
    return maps


def kernel(hidden_states, Wq, Wk, Wv, Wo, idx_wq, idx_wk):
    from concourse.bass_utils import run_bass_kernel_spmd

    if "nc" not in _COMPILED:
        _COMPILED["nc"] = _build_module()
    nc = _COMPILED["nc"]

    in_maps = _prep_inputs(np.asarray(hidden_states), np.asarray(Wq),
                           np.asarray(Wk), np.asarray(Wv), np.asarray(Wo),
                           np.asarray(idx_wq), np.asarray(idx_wk))
    res = run_bass_kernel_spmd(nc, in_maps, core_ids=list(range(NCORES)))
    out = np.zeros((T, D), np.float32)
    for c in range(NCORES):
        out += np.asarray(res.results[c]["out_part"], np.float32)
    return out.reshape(B, T, D)
